# revision 1
# baseline (speedup 1.0000x reference)
"""CorefModel TRN2 kernel: 8-core SPMD Bass implementation.

Pipeline (3 device launches + host glue):
  L1: span scores  [C=24576] -- candidate-span axis sharded over 8 cores.
      Decomposed span FFNN: score = w1.relu(A[s] + B[s+w] + Wd[w] + b0
      + (sum_t p_t G[t]) / den), with A/B/G = te @ W0-slices computed once
      per token instead of per span (16x FLOP reduction).
  host: mask invalid, stable argsort, greedy non-crossing NMS -> top_idx.
  L2: top_emb assembly (start/end/head/width, D padded to 2432) + src =
      emb @ c2f + b, partial pair matrix (contraction-d sharded; host sums).
  host: visibility mask + top_score add, per-row stable top-k -> ante_idx,
      distance buckets.
  L3: pair FFNN slow score, pairs sharded over cores; gamma term
      (emb_i*emb_j) @ W0C in bf16, alpha/beta terms via fp32 gathers.

All selection-critical arithmetic (L1 scores, L2 pair matrix) is fp32;
only the slow-score value path uses bf16.
"""
import numpy as np
import ml_dtypes
from contextlib import ExitStack

import concourse.bass as bass
import concourse.mybir as mybir
import concourse.tile as tile
from concourse import bacc
from concourse.bass_utils import run_bass_kernel_spmd
from concourse.masks import make_identity

dt = mybir.dt
AF = mybir.ActivationFunctionType
ALU = mybir.AluOpType

NCORES = 8
T, E, W, F, H = 1536, 768, 16, 20, 1000
D = 2 * E + F + E              # 2324
DP = 2432                      # D padded to 19*128 (feature order: start,end,head,width,pad)
KC = DP // 128                 # 19 d-chunks
TC = T // NCORES               # 192 starts per core
COLS = 256                     # per-core t-window (t0 .. t0+256), te padded to 1600
TPAD = T + 64                  # 1600
SPC = TC * W                   # 3072 spans per core
M_CAP, K_CAP = 300, 50
MP = 304                       # m padded to 8*38
IPC = MP // NCORES             # 38 i-rows per core
PPC = 1920                     # padded pairs per core (38*50=1900 -> 15*128)
PCH = PPC // 128               # 15 pair chunks
NEG = np.float32(-1e30)

# se' feature permutation: old order [start(768) end(768) width(20) head(768)],
# new order [start end head width pad(108)]
_PERM = np.concatenate([
    np.arange(0, 2 * E),                   # start, end
    np.arange(2 * E + F, 2 * E + F + E),   # head
    np.arange(2 * E, 2 * E + F),           # width
]).astype(np.int64)            # maps new[0:2324] -> old index


def _pad_rows(a, rows):
    out = np.zeros((rows,) + a.shape[1:], a.dtype)
    out[:a.shape[0]] = a
    return out


# ---------------------------------------------------------------- L1 ----
def build_l1():
    nc = bacc.Bacc(trn_type="TRN2", target_bir_lowering=False, debug=False)
    te_loc = nc.dram_tensor("te_loc", [COLS, E], dt.float32, kind="ExternalInput").ap()
    p_loc = nc.dram_tensor("p_loc", [1, COLS], dt.float32, kind="ExternalInput").ap()
    r_loc = nc.dram_tensor("r_loc", [1, SPC], dt.float32, kind="ExternalInput").ap()
    w0abd = nc.dram_tensor("w0abd", [8, 18, 128, H // 8], dt.float32, kind="ExternalInput").ap()
    w0wd = nc.dram_tensor("w0wd", [F + 1, H], dt.float32, kind="ExternalInput").ap()
    we_aug = nc.dram_tensor("we_aug", [F + 1, W], dt.float32, kind="ExternalInput").ap()
    w1_in = nc.dram_tensor("w1_in", [H, 1], dt.float32, kind="ExternalInput").ap()
    score_out = nc.dram_tensor("score", [1, SPC], dt.float32, kind="ExternalOutput").ap()

    HC = 8
    HB = H // HC  # 125
    NT = 512

    with tile.TileContext(nc) as tc, ExitStack() as ctx:
        cst = ctx.enter_context(tc.tile_pool(name="cst", bufs=1))
        ps = ctx.enter_context(tc.tile_pool(name="ps", bufs=2, space="PSUM"))
        scps = ctx.enter_context(tc.tile_pool(name="scps", bufs=1, space="PSUM"))
        npool = ctx.enter_context(tc.tile_pool(name="npool", bufs=3))
        abp = ctx.enter_context(tc.tile_pool(name="abp", bufs=3))
        comb = ctx.enter_context(tc.tile_pool(name="comb", bufs=3))

        # --- constant loads (G-part of W first so PE starts early)
        te_sb = cst.tile([128, 2, E], dt.float32, tag="te_sb")
        nc.sync.dma_start(out=te_sb[:], in_=te_loc.rearrange("(c p) e -> p c e", p=128))
        ident = cst.tile([128, 128], dt.float32, tag="ident")
        make_identity(nc, ident)
        wdsb = cst.tile([F + 1, H], dt.float32, tag="wdsb")
        nc.sync.dma_start(out=wdsb[:], in_=w0wd[:, :])
        wesb = cst.tile([F + 1, W], dt.float32, tag="wesb")
        nc.sync.dma_start(out=wesb[:], in_=we_aug[:, :])
        w1sb = cst.tile([HB, HC], dt.float32, tag="w1sb")
        nc.sync.dma_start(out=w1sb[:], in_=w1_in.rearrange("(c p) one -> p (c one)", p=HB))
        p_bc = cst.tile([128, COLS], dt.float32, tag="p_bc")
        nc.sync.dma_start(out=p_bc[:], in_=p_loc[0:1, :].broadcast_to([128, COLS]))
        r_bc = cst.tile([128, SPC], dt.float32, tag="r_bc")
        nc.sync.dma_start(out=r_bc[:], in_=r_loc[0:1, :].broadcast_to([128, SPC]))

        # --- te^T tiles [128e, 256t] x6
        teT = []
        for ec in range(6):
            t_ = cst.tile([128, COLS], dt.float32, tag=f"teT{ec}", name=f"teT{ec}")
            teT.append(t_)
        for tcki in range(2):
            for ec in range(6):
                pt = ps.tile([128, 128], dt.float32, tag="acc")
                nc.tensor.transpose(out=pt[:], in_=te_sb[:, tcki, ec * 128:(ec + 1) * 128],
                                    identity=ident[:])
                nc.scalar.copy(teT[ec][:, tcki * 128:(tcki + 1) * 128], pt[:])

        sc_ps = [scps.tile([1, NT], dt.float32, tag=f"sc{j}", name=f"sc{j}")
                 for j in range(SPC // NT)]

        wpool = ctx.enter_context(tc.tile_pool(name="wpool", bufs=3))

        def mat_tile(whc, m3, tag):
            t_ = abp.tile([HB, COLS], dt.float32, tag=tag, name=tag)
            acc = ps.tile([HB, COLS], dt.float32, tag="acc", name="acc")
            for ec in range(6):
                nc.tensor.matmul(acc[:],
                                 lhsT=whc[:, m3 * 6 + ec, :],
                                 rhs=teT[ec][:],
                                 start=(ec == 0), stop=(ec == 5))
            nc.scalar.copy(t_[:], acc[:])
            return t_

        # --- per-h-chunk pipeline (W streamed per h-chunk, G chunks first)
        for hc in range(HC):
            whc = wpool.tile([128, 18, HB], dt.float32, tag="whc", name="whc")
            nc.sync.dma_start(out=whc[:], in_=w0abd[hc].rearrange("c p h -> p c h"))
            g_t = mat_tile(whc, 2, "g_t")
            pg = npool.tile([HB, COLS], dt.float32, tag="pg", name="pg")
            nc.vector.tensor_mul(pg[:], g_t[:], p_bc[:HB, :])
            numw = npool.tile([HB, SPC], dt.float32, tag="numw", name="numw")
            n3 = numw[:].rearrange("p (s w) -> p s w", w=W)
            nc.gpsimd.tensor_copy(n3[:, :, 0], pg[:, 0:TC])
            for w_ in range(1, W):
                nc.gpsimd.tensor_add(n3[:, :, w_], n3[:, :, w_ - 1], pg[:, w_:w_ + TC])
            a_t = mat_tile(whc, 0, "a_t")
            b_t = mat_tile(whc, 1, "b_t")
            wd_ps = ps.tile([HB, W], dt.float32, tag="acc", name="wdacc")
            nc.tensor.matmul(wd_ps[:], lhsT=wdsb[:, hc * HB:(hc + 1) * HB],
                             rhs=wesb[:], start=True, stop=True)
            wd_t = abp.tile([HB, W], dt.float32, tag="wd_t", name="wd_t")
            nc.scalar.copy(wd_t[:], wd_ps[:])

            # full-width combine: x = numW*r + A + B  (three wide DVE ops)
            x = comb.tile([HB, SPC], dt.float32, tag="x", name="x")
            nc.vector.tensor_mul(x[:], numw[:], r_bc[:HB, :])
            x3 = x[:].rearrange("p (s w) -> p s w", w=W)
            a_op = a_t[:, 0:TC][:, :, None].broadcast_to([HB, TC, W])
            nc.vector.tensor_add(x3, x3, a_op)
            bt = b_t[:]
            b_op = bass.AP(bt.tensor, bt.offset, [bt.ap[0], [1, TC], [1, W]])
            nc.vector.tensor_add(x3, x3, b_op)
            # wd folded into relu bias, one ACT op per width
            y = comb.tile([HB, SPC], dt.float32, tag="y", name="y")
            y3 = y[:].rearrange("p (s w) -> p s w", w=W)
            for w_ in range(W):
                nc.scalar.activation(y3[:, :, w_], x3[:, :, w_], AF.Relu,
                                     bias=wd_t[:, w_:w_ + 1])
            for j in range(SPC // NT):
                nc.tensor.matmul(sc_ps[j][:], lhsT=w1sb[:, hc:hc + 1],
                                 rhs=y[:, j * NT:(j + 1) * NT],
                                 start=(hc == 0), stop=(hc == HC - 1))

        for j in range(SPC // NT):
            so = comb.tile([1, NT], dt.float32, tag="so", name="so")
            nc.vector.tensor_copy(so[:], sc_ps[j][:])
            nc.sync.dma_start(out=score_out[:, j * NT:(j + 1) * NT], in_=so[:])

    nc.compile()
    return nc


# ---------------------------------------------------------------- L2 ----
def build_l2():
    nc = bacc.Bacc(trn_type="TRN2", target_bir_lowering=False, debug=False)
    te_pad = nc.dram_tensor("te_pad", [TPAD, E], dt.float32, kind="ExternalInput").ap()
    iota_c = nc.dram_tensor("iota_c", [128, 12], dt.float32, kind="ExternalInput").ap()
    p_row = nc.dram_tensor("p_row", [1, T], dt.float32, kind="ExternalInput").ap()
    s_row = nc.dram_tensor("s_row", [1, M_CAP], dt.float32, kind="ExternalInput").ap()
    e_row = nc.dram_tensor("e_row", [1, M_CAP], dt.float32, kind="ExternalInput").ap()
    r300 = nc.dram_tensor("r300", [1, M_CAP], dt.float32, kind="ExternalInput").ap()
    sidx = nc.dram_tensor("sidx", [384, 1], dt.int32, kind="ExternalInput").ap()
    eidx = nc.dram_tensor("eidx", [384, 1], dt.int32, kind="ExternalInput").ap()
    we_sel = nc.dram_tensor("we_sel", [F, M_CAP], dt.float32, kind="ExternalInput").ap()
    c2f_c = nc.dram_tensor("c2f_c", [DP, 384], dt.float32, kind="ExternalInput").ap()
    c2fb_c = nc.dram_tensor("c2fb_c", [1, 384], dt.float32, kind="ExternalInput").ap()
    w0a_c = nc.dram_tensor("w0a_c", [DP, 125], dt.float32, kind="ExternalInput").ap()
    w0b_c = nc.dram_tensor("w0b_c", [DP, 125], dt.float32, kind="ExternalInput").ap()
    chrows = nc.dram_tensor("chrows", [128, 3], dt.int32, kind="ExternalInput").ap()

    pair_out = nc.dram_tensor("pair_part", [M_CAP, M_CAP], dt.float32, kind="ExternalOutput").ap()
    emb_out = nc.dram_tensor("emb_rows", [M_CAP, DP], dt.float32, kind="ExternalOutput").ap()
    a_out = nc.dram_tensor("alpha_part", [M_CAP, 125], dt.float32, kind="ExternalOutput").ap()
    b_out = nc.dram_tensor("beta_part", [M_CAP, 125], dt.float32, kind="ExternalOutput").ap()

    MM = M_CAP
    with tile.TileContext(nc) as tc, ExitStack() as ctx:
        cst = ctx.enter_context(tc.tile_pool(name="cst", bufs=1))
        ps = ctx.enter_context(tc.tile_pool(name="ps", bufs=2, space="PSUM"))
        dpool = ctx.enter_context(tc.tile_pool(name="dpool", bufs=1, space="DRAM"))
        embt_scr = dpool.tile([DP, M_CAP], dt.float32, tag="embt_scr")

        ident = cst.tile([128, 128], dt.float32, tag="ident")
        make_identity(nc, ident)
        ones1 = cst.tile([1, MM], dt.float32, tag="ones1")
        nc.vector.memset(ones1[:], 1.0)
        iot = cst.tile([128, 12], dt.float32, tag="iot")
        nc.sync.dma_start(out=iot[:], in_=iota_c[:, :])
        s_bc = cst.tile([128, MM], dt.float32, tag="s_bc")
        nc.sync.dma_start(out=s_bc[:], in_=s_row[0:1, :].broadcast_to([128, MM]))
        e_bc = cst.tile([128, MM], dt.float32, tag="e_bc")
        nc.sync.dma_start(out=e_bc[:], in_=e_row[0:1, :].broadcast_to([128, MM]))
        r_bc = cst.tile([128, MM], dt.float32, tag="r_bc")
        nc.sync.dma_start(out=r_bc[:], in_=r300[0:1, :].broadcast_to([128, MM]))
        p_sb = cst.tile([128, 12], dt.float32, tag="p_sb")
        nc.sync.dma_start(out=p_sb[:], in_=p_row.rearrange("one (c p) -> p (one c)", p=128))
        wesel = cst.tile([F, MM], dt.float32, tag="wesel")
        nc.sync.dma_start(out=wesel[:], in_=we_sel[:, :])
        sidxt = cst.tile([128, 3], dt.int32, tag="sidxt")
        nc.sync.dma_start(out=sidxt[:], in_=sidx.rearrange("(g p) one -> p (g one)", p=128))
        eidxt = cst.tile([128, 3], dt.int32, tag="eidxt")
        nc.sync.dma_start(out=eidxt[:], in_=eidx.rearrange("(g p) one -> p (g one)", p=128))
        chrt = cst.tile([128, 3], dt.int32, tag="chrt")
        nc.sync.dma_start(out=chrt[:], in_=chrows[:, :])
        te_ch = cst.tile([128, 12, E], dt.float32, tag="te_ch")
        te_pad3 = te_pad[:T].rearrange("(c p) e -> p c e", p=128)
        for ck in range(12):
            nc.sync.dma_start(out=te_ch[:, ck, :], in_=te_pad3[:, ck, :])
        c2f_sb = cst.tile([128, KC, 384], dt.float32, tag="c2f_sb")
        c2f3 = c2f_c.rearrange("(c p) s -> p c s", p=128)
        for ck in range(KC):
            nc.sync.dma_start(out=c2f_sb[:, ck, :], in_=c2f3[:, ck, :])
        c2fb_sb = cst.tile([1, 384], dt.float32, tag="c2fb_sb")
        nc.sync.dma_start(out=c2fb_sb[:], in_=c2fb_c[:, :])
        w0a_sb = cst.tile([128, KC, 125], dt.float32, tag="w0a_sb")
        nc.sync.dma_start(out=w0a_sb[:], in_=w0a_c.rearrange("(c p) s -> p c s", p=128))
        w0b_sb = cst.tile([128, KC, 125], dt.float32, tag="w0b_sb")
        nc.sync.dma_start(out=w0b_sb[:], in_=w0b_c.rearrange("(c p) s -> p c s", p=128))

        # --- embT chunks [128, 300] x19
        embT = []
        for kc in range(KC):
            t_ = cst.tile([128, MM], dt.float32, tag=f"embT{kc}", name=f"embT{kc}")
            embT.append(t_)

        # start/end cols: gather rows then transpose into chunks 0-5 / 6-11
        gath = ctx.enter_context(tc.tile_pool(name="gath", bufs=2))
        for part, idxt in ((0, sidxt), (1, eidxt)):
            for g in range(3):
                gn = min(128, MM - g * 128)
                rows = gath.tile([128, E], dt.float32, tag="rows")
                nc.gpsimd.indirect_dma_start(
                    out=rows[:gn, :], out_offset=None, in_=te_pad[:, :],
                    in_offset=bass.IndirectOffsetOnAxis(ap=idxt[:gn, g:g + 1], axis=0))
                for ec in range(6):
                    pt = ps.tile([128, 128], dt.float32, tag="tr_ps")
                    nc.tensor.transpose(out=pt[:, :gn], in_=rows[:gn, ec * 128:(ec + 1) * 128],
                                        identity=ident[:gn, :gn])
                    nc.scalar.copy(embT[part * 6 + ec][:, g * 128:g * 128 + gn], pt[:, :gn])

        # head: mask M[t, j] = p_t * (s_j <= t <= e_j), num = te^T @ M, * r
        msk = []
        for tck in range(12):
            t_ = cst.tile([128, MM], dt.float32, tag=f"msk{tck}", name=f"msk{tck}")
            msk.append(t_)
        mtmp = ctx.enter_context(tc.tile_pool(name="mtmp", bufs=2))
        for tck in range(12):
            mge = mtmp.tile([128, MM], dt.float32, tag="mge")
            nc.vector.tensor_scalar(mge[:], s_bc[:], iot[:, tck:tck + 1], None, ALU.is_le)
            mle = mtmp.tile([128, MM], dt.float32, tag="mle")
            nc.vector.tensor_scalar(mle[:], e_bc[:], iot[:, tck:tck + 1], None, ALU.is_ge)
            nc.vector.scalar_tensor_tensor(msk[tck][:], mge[:], p_sb[:, tck:tck + 1],
                                           mle[:], ALU.mult, ALU.mult)
        for ec in range(6):
            acc = ps.tile([128, MM], dt.float32, tag="acc_ps")
            for tck in range(12):
                nc.tensor.matmul(acc[:], lhsT=te_ch[:, tck, ec * 128:(ec + 1) * 128],
                                 rhs=msk[tck][:], start=(tck == 0), stop=(tck == 11))
            nc.vector.tensor_mul(embT[12 + ec][:], acc[:], r_bc[:])

        # width chunk 18: [we_sel(20); zeros(108)]
        nc.vector.memset(embT[18][:], 0.0)
        nc.vector.tensor_copy(embT[18][0:F, :], wesel[:])

        # write embT scratch + emb rows (transposed) outputs
        for kc in range(KC):
            nc.sync.dma_start(out=embt_scr[kc * 128:(kc + 1) * 128, :], in_=embT[kc][:])
        erow = ctx.enter_context(tc.tile_pool(name="erow", bufs=2))
        for g in range(3):
            gn = min(128, MM - g * 128)
            rowt = erow.tile([128, DP], dt.float32, tag="rowt")
            for kc in range(KC):
                pt = ps.tile([128, 128], dt.float32, tag="tr_ps")
                nc.tensor.transpose(out=pt[:gn, :], in_=embT[kc][:, g * 128:g * 128 + gn],
                                    identity=ident[:])
                nc.scalar.copy(rowt[:gn, kc * 128:(kc + 1) * 128], pt[:gn, :])
            nc.sync.dma_start(out=emb_out[g * 128:g * 128 + gn, :], in_=rowt[:gn, :])

        # --- src slots + partial pair
        slotp = ctx.enter_context(tc.tile_pool(name="slotp", bufs=1))
        srcT = []
        for si in range(3):
            t_ = slotp.tile([128, MM], dt.float32, tag=f"srcT{si}", name=f"srcT{si}")
            srcT.append(t_)
        esel = []
        for si in range(3):
            t_ = slotp.tile([128, MM], dt.float32, tag=f"esel{si}", name=f"esel{si}")
            esel.append(t_)
        for si in range(3):
            acc = ps.tile([128, MM], dt.float32, tag="acc_ps")
            nc.tensor.matmul(acc[:], lhsT=c2fb_sb[:, si * 128:(si + 1) * 128],
                             rhs=ones1[:], start=True, stop=False)
            for kc in range(KC):
                nc.tensor.matmul(acc[:], lhsT=c2f_sb[:, kc, si * 128:(si + 1) * 128],
                                 rhs=embT[kc][:], start=False, stop=(kc == KC - 1))
            nc.scalar.copy(srcT[si][:], acc[:])
            nc.gpsimd.indirect_dma_start(
                out=esel[si][:], out_offset=None, in_=embt_scr[:, :],
                in_offset=bass.IndirectOffsetOnAxis(ap=chrt[:, si:si + 1], axis=0))
        prow = ctx.enter_context(tc.tile_pool(name="prow", bufs=2))
        for it in range(3):
            inn = min(128, MM - it * 128)
            acc = ps.tile([128, MM], dt.float32, tag="acc_ps")
            for si in range(3):
                nc.tensor.matmul(acc[:inn, :], lhsT=srcT[si][:, it * 128:it * 128 + inn],
                                 rhs=esel[si][:], start=(si == 0), stop=(si == 2))
            pr = prow.tile([128, MM], dt.float32, tag="pr")
            nc.scalar.copy(pr[:inn, :], acc[:inn, :])
            nc.sync.dma_start(out=pair_out[it * 128:it * 128 + inn, :], in_=pr[:inn, :])

        # --- alpha/beta h-chunk (125 cols of H per core)
        abrow = ctx.enter_context(tc.tile_pool(name="abrow", bufs=2))
        for wsb_, out_ in ((w0a_sb, a_out), (w0b_sb, b_out)):
            acc = ps.tile([125, MM], dt.float32, tag="acc_ps")
            for kc in range(KC):
                nc.tensor.matmul(acc[:], lhsT=wsb_[:, kc, :], rhs=embT[kc][:],
                                 start=(kc == 0), stop=(kc == KC - 1))
            abT = abrow.tile([125, MM], dt.float32, tag="abT")
            nc.scalar.copy(abT[:], acc[:])
            for g in range(3):
                gn = min(128, MM - g * 128)
                pt = ps.tile([128, 125], dt.float32, tag="tr_ps")
                nc.tensor.transpose(out=pt[:gn, :], in_=abT[:, g * 128:g * 128 + gn],
                                    identity=ident[:125, :125])
                ab_r = abrow.tile([128, 125], dt.float32, tag="ab_r")
                nc.scalar.copy(ab_r[:gn, :], pt[:gn, :])
                nc.sync.dma_start(out=out_[g * 128:g * 128 + gn, :], in_=ab_r[:gn, :])

    nc.compile()
    return nc


# ---------------------------------------------------------------- L3 ----
def build_l3():
    nc = bacc.Bacc(trn_type="TRN2", target_bir_lowering=False, debug=False)
    embb = nc.dram_tensor("embb", [MP, DP], dt.bfloat16, kind="ExternalInput").ap()
    iidx = nc.dram_tensor("iidx", [PPC, 1], dt.int32, kind="ExternalInput").ap()
    alpha = nc.dram_tensor("alpha", [MP, H], dt.float32, kind="ExternalInput").ap()
    jidx = nc.dram_tensor("jidx", [PPC, 1], dt.int32, kind="ExternalInput").ap()
    beta = nc.dram_tensor("beta", [MP, H], dt.float32, kind="ExternalInput").ap()
    dembt = nc.dram_tensor("dembt", [F + 1, PPC], dt.bfloat16, kind="ExternalInput").ap()
    w0c = nc.dram_tensor("w0c", [DP, H], dt.bfloat16, kind="ExternalInput").ap()
    w0d2 = nc.dram_tensor("w0d2", [F + 1, H], dt.bfloat16, kind="ExternalInput").ap()
    w1row = nc.dram_tensor("w1row", [1, H], dt.float32, kind="ExternalInput").ap()
    slow_out = nc.dram_tensor("slow", [128, PCH], dt.float32, kind="ExternalOutput").ap()

    NH = 500
    with tile.TileContext(nc) as tc, ExitStack() as ctx:
        cst = ctx.enter_context(tc.tile_pool(name="cst", bufs=1))
        w0c_sb = cst.tile([128, KC, H], dt.bfloat16, tag="w0c_sb")
        nc.sync.dma_start(out=w0c_sb[:], in_=w0c.rearrange("(c p) h -> p c h", p=128))
        w0d2_sb = cst.tile([F + 1, H], dt.bfloat16, tag="w0d2_sb")
        nc.sync.dma_start(out=w0d2_sb[:], in_=w0d2[:, :])
        demb_sb = cst.tile([F + 1, PPC], dt.bfloat16, tag="demb_sb")
        nc.sync.dma_start(out=demb_sb[:], in_=dembt[:, :])
        w1_bc = cst.tile([128, H], dt.float32, tag="w1_bc")
        nc.sync.dma_start(out=w1_bc[:], in_=w1row[0:1, :].broadcast_to([128, H]))
        iidx_sb = cst.tile([128, PCH], dt.int32, tag="iidx_sb")
        nc.sync.dma_start(out=iidx_sb[:], in_=iidx.rearrange("(c p) one -> p (c one)", p=128))
        jidx_sb = cst.tile([128, PCH], dt.int32, tag="jidx_sb")
        nc.sync.dma_start(out=jidx_sb[:], in_=jidx.rearrange("(c p) one -> p (c one)", p=128))
        slow_sb = cst.tile([128, PCH], dt.float32, tag="slow_sb")

        wk = ctx.enter_context(tc.tile_pool(name="wk", bufs=3))
        zt_pool = ctx.enter_context(tc.tile_pool(name="zt", bufs=3))
        ps = ctx.enter_context(tc.tile_pool(name="ps", bufs=2, space="PSUM"))

        for pc in range(PCH):
            ei = wk.tile([128, DP], dt.bfloat16, tag="ei")
            nc.gpsimd.indirect_dma_start(
                out=ei[:], out_offset=None, in_=embb[:, :],
                in_offset=bass.IndirectOffsetOnAxis(ap=iidx_sb[:, pc:pc + 1], axis=0))
            ej = wk.tile([128, DP], dt.bfloat16, tag="ej")
            nc.gpsimd.indirect_dma_start(
                out=ej[:], out_offset=None, in_=embb[:, :],
                in_offset=bass.IndirectOffsetOnAxis(ap=jidx_sb[:, pc:pc + 1], axis=0))
            z = wk.tile([128, DP], dt.bfloat16, tag="z")
            nc.vector.tensor_mul(z[:], ei[:], ej[:])
            zT = zt_pool.tile([128, KC, 128], dt.bfloat16, tag="zT")
            nc.sync.dma_start_transpose(zT[:], z[:])

            ab = wk.tile([128, H], dt.float32, tag="ab")
            nc.gpsimd.indirect_dma_start(
                out=ab[:], out_offset=None, in_=beta[:, :],
                in_offset=bass.IndirectOffsetOnAxis(ap=jidx_sb[:, pc:pc + 1], axis=0))
            aa = wk.tile([128, H], dt.float32, tag="aa")
            nc.gpsimd.indirect_dma_start(
                out=aa[:], out_offset=None, in_=alpha[:, :],
                in_offset=bass.IndirectOffsetOnAxis(ap=iidx_sb[:, pc:pc + 1], axis=0))

            x = wk.tile([128, H], dt.float32, tag="xcomb")
            for hh in range(2):
                acc = ps.tile([128, NH], dt.float32, tag="g_ps")
                nc.tensor.matmul(acc[:], lhsT=demb_sb[:, pc * 128:(pc + 1) * 128],
                                 rhs=w0d2_sb[:, hh * NH:(hh + 1) * NH],
                                 start=True, stop=False)
                for kc in range(KC):
                    nc.tensor.matmul(acc[:], lhsT=zT[:, kc, :],
                                     rhs=w0c_sb[:, kc, hh * NH:(hh + 1) * NH],
                                     start=False, stop=(kc == KC - 1))
                nc.vector.tensor_add(x[:, hh * NH:(hh + 1) * NH], acc[:],
                                     ab[:, hh * NH:(hh + 1) * NH])
            nc.gpsimd.tensor_add(x[:], x[:], aa[:])
            y = wk.tile([128, H], dt.float32, tag="ystt")
            nc.vector.scalar_tensor_tensor(
                y[:], x[:], 0.0, w1_bc[:],
                ALU.max, ALU.mult, accum_out=slow_sb[:, pc:pc + 1])
        nc.sync.dma_start(out=slow_out[:, :], in_=slow_sb[:])

    nc.compile()
    return nc


# ------------------------------------------------------------- host -----
_BUILT = {}
TRACE = False
PERF = {}


def _run(name, nc, in_maps, cores):
    try:
        res = run_bass_kernel_spmd(nc, in_maps, core_ids=cores, trace=TRACE)
    except ModuleNotFoundError:
        # axon NTFF profiling hook unavailable in this container
        res = run_bass_kernel_spmd(nc, in_maps, core_ids=cores)
    if res.exec_time_ns is not None:
        PERF[name] = res.exec_time_ns
    return res


def _get(name, builder):
    if name not in _BUILT:
        _BUILT[name] = builder()
    return _BUILT[name]


def _bucket_dist(d):
    # replicate reference _bucket_distance in fp32
    df = d.astype(np.float32)
    with np.errstate(divide="ignore"):
        log_idx = np.floor(np.log(np.maximum(df, np.float32(1.0))) /
                           np.float32(np.log(2.0))) + np.float32(3.0)
    comb = np.where(d <= 4, df, log_idx)
    return np.clip(comb, 0, 9).astype(np.int32)


def _nms(order, starts, ends, m):
    top = []
    ts = np.zeros(m, np.int64)
    te_ = np.zeros(m, np.int64)
    n = 0
    i = 0
    C = order.shape[0]
    while n < m and i < C:
        idx = order[i]
        s, e = starts[idx], ends[idx]
        if n:
            a = ts[:n]
            b = te_[:n]
            cross = (((s < a) & (e < b) & (e >= a)) | ((s > a) & (s <= b) & (e > b))).any()
        else:
            cross = False
        if not cross:
            top.append(idx)
            ts[n] = s
            te_[n] = e
            n += 1
        i += 1
    return np.asarray(top, np.int64)


def kernel(tokens_embed, Wh_w, Wh_b, width_emb, dist_emb,
           Sm_w0, Sm_b0, Sm_w1, Sm_b1,
           Smm_w0, Smm_b0, Smm_w1, Smm_b1,
           c2f_w, c2f_b, m, k):
    m = int(m)
    k = int(k)
    assert m == M_CAP and k == K_CAP, (m, k)
    f32 = np.float32
    te = np.ascontiguousarray(tokens_embed, f32)
    cores = list(range(NCORES))

    # ---- host prep (index/pad/exp glue)
    te_pad = np.concatenate([te, np.repeat(te[-1:], TPAD - T, 0)], 0)
    tok = (te @ Wh_w.astype(f32))[:, 0] + f32(Wh_b[0])
    p_full = np.exp(tok.astype(f32)).astype(f32)
    p_pad = np.concatenate([p_full, np.zeros(TPAD - T, f32)])
    # den/r per (s, w): sequential fp32 prefix
    den = np.empty((T, W), f32)
    acc = p_full.copy()
    den[:, 0] = acc
    for w_ in range(1, W):
        nxt = np.concatenate([p_full[w_:], np.zeros(w_, f32)])
        acc = (acc + nxt).astype(f32)
        den[:, w_] = acc
    r_full = (f32(1.0) / den).astype(f32)

    # L1 inputs
    w0abd_flat = np.concatenate([Sm_w0[0:E], Sm_w0[E:2 * E], Sm_w0[2 * E + F:]], 0).astype(f32)
    w0abd = np.ascontiguousarray(
        w0abd_flat.reshape(18, 128, 8, 125).transpose(2, 0, 1, 3))
    w0wd = np.concatenate([Sm_w0[2 * E:2 * E + F], Sm_b0[None, :]], 0).astype(f32)
    we_aug = np.concatenate([width_emb.T, np.ones((1, W))], 0).astype(f32)
    in1 = []
    for c in cores:
        t0 = c * TC
        in1.append(dict(
            te_loc=np.ascontiguousarray(te_pad[t0:t0 + COLS]),
            p_loc=np.ascontiguousarray(p_pad[None, t0:t0 + COLS]),
            r_loc=np.ascontiguousarray(r_full[t0:t0 + TC].reshape(1, SPC)),
            w0abd=w0abd, w0wd=w0wd, we_aug=we_aug,
            w1_in=Sm_w1.astype(f32)))
    nc1 = _get("l1", build_l1)
    res1 = _run("l1", nc1, in1, cores)
    scores = np.concatenate([res1.results[c]["score"][0] for c in cores])

    # ---- host: mask + NMS
    starts = np.repeat(np.arange(T, dtype=np.int64), W)
    widths = np.tile(np.arange(W, dtype=np.int64), T)
    ends = starts + widths
    valid = ends < T
    sc = np.where(valid, (scores + f32(Sm_b1[0])).astype(f32), NEG).astype(f32)
    order = np.argsort(-sc, kind="stable")
    top_idx = _nms(order, starts, ends, m)
    ts_, tw = starts[top_idx], widths[top_idx]
    tec = np.minimum(ts_ + tw, T - 1)
    top_score = sc[top_idx]

    # ---- L2
    perm = _PERM
    c2f_pad = np.zeros((DP, DP), f32)
    c2f_pad[:D, :D] = c2f_w.astype(f32)[np.ix_(perm, perm)]
    c2fb_pad = np.zeros(DP, f32)
    c2fb_pad[:D] = c2f_b.astype(f32)[perm]
    w0a_pad = np.zeros((DP, H), f32)
    w0a_pad[:D] = Smm_w0[0:D].astype(f32)[perm]
    w0b_pad = np.zeros((DP, H), f32)
    w0b_pad[:D] = Smm_w0[D:2 * D].astype(f32)[perm]
    iota_c = (np.arange(128)[:, None] + 128 * np.arange(12)[None, :]).astype(f32)
    we_selv = width_emb.T.astype(f32)[:, tw]
    r300v = r_full[ts_, tw][None, :].astype(f32)
    chunk_sets = [list(range(3 * c, 3 * c + 3)) for c in cores]
    in2 = []
    for c in cores:
        cs = chunk_sets[c]
        c2fc = np.zeros((DP, 384), f32)
        c2fbc = np.zeros((1, 384), f32)
        chr_ = np.zeros((128, 3), np.int32)
        for si, ch in enumerate(cs):
            if ch < KC:
                c2fc[:, si * 128:(si + 1) * 128] = c2f_pad[:, ch * 128:(ch + 1) * 128]
                c2fbc[0, si * 128:(si + 1) * 128] = c2fb_pad[ch * 128:(ch + 1) * 128]
                chr_[:, si] = ch * 128 + np.arange(128)
        hc0 = c * 125
        in2.append(dict(
            te_pad=te_pad, iota_c=iota_c, p_row=p_full[None, :],
            s_row=ts_[None, :].astype(f32), e_row=np.minimum(ts_ + tw, T - 1)[None, :].astype(f32),
            r300=r300v, sidx=_pad_rows(ts_[:, None].astype(np.int32), 384),
            eidx=_pad_rows(tec[:, None].astype(np.int32), 384),
            we_sel=we_selv, c2f_c=c2fc, c2fb_c=c2fbc,
            w0a_c=np.ascontiguousarray(w0a_pad[:, hc0:hc0 + 125]),
            w0b_c=np.ascontiguousarray(w0b_pad[:, hc0:hc0 + 125]),
            chrows=chr_))
    nc2 = _get("l2", build_l2)
    res2 = _run("l2", nc2, in2, cores)
    pair = np.zeros((m, m), f32)
    for c in cores:
        pair += res2.results[c]["pair_part"]
    pair = pair.astype(f32)
    emb_rows = res2.results[0]["emb_rows"]
    alpha = np.concatenate([res2.results[c]["alpha_part"] for c in cores], 1)
    beta = np.concatenate([res2.results[c]["beta_part"] for c in cores], 1)

    # ---- host: visibility + top-k
    offset = np.arange(m, dtype=np.int64)[:, None] - np.arange(m, dtype=np.int64)[None, :]
    vis = offset >= 1
    all_score = (np.where(vis, f32(0.0), NEG).astype(f32) + top_score[:, None]).astype(f32)
    all_score = (all_score + pair).astype(f32)
    ante_idx = np.argsort(-all_score, axis=1, kind="stable")[:, :k]
    fast = np.take_along_axis(all_score, ante_idx, axis=1).astype(f32)
    ante_off = np.take_along_axis(offset, ante_idx, axis=1)
    dbuck = _bucket_dist(ante_off)
    demb_pairs = dist_emb.astype(f32)[dbuck]            # [m, k, F]

    # ---- L3
    bf16 = ml_dtypes.bfloat16
    emb_mp = _pad_rows(emb_rows, MP)
    embb = emb_mp.astype(bf16)
    alpha_mp = _pad_rows(alpha.astype(f32), MP)
    beta_mp = _pad_rows(beta.astype(f32), MP)
    w0c_pad = np.zeros((DP, H), f32)
    w0c_pad[:D] = Smm_w0[2 * D:3 * D].astype(f32)[perm]
    w0d2 = np.concatenate([Smm_w0[3 * D:3 * D + F].astype(f32), Smm_b0[None, :].astype(f32)], 0)
    in3 = []
    for c in cores:
        i0 = c * IPC
        ii = np.zeros((PPC, 1), np.int32)
        jj = np.zeros((PPC, 1), np.int32)
        dmb = np.zeros((F + 1, PPC), f32)
        for r_ in range(IPC):
            gi = i0 + r_
            if gi < m:
                sl = slice(r_ * K_CAP, (r_ + 1) * K_CAP)
                ii[sl, 0] = gi
                jj[sl, 0] = ante_idx[gi]
                dmb[:F, sl] = demb_pairs[gi].T
                dmb[F, sl] = 1.0
        in3.append(dict(
            embb=embb, iidx=ii, jidx=jj, alpha=alpha_mp, beta=beta_mp,
            dembt=dmb.astype(bf16), w0c=w0c_pad.astype(bf16),
            w0d2=w0d2.astype(bf16), w1row=np.ascontiguousarray(Smm_w1.astype(f32).T)))
    nc3 = _get("l3", build_l3)
    res3 = _run("l3", nc3, in3, cores)
    slow = np.zeros((m, k), f32)
    for c in cores:
        sl = np.ascontiguousarray(res3.results[c]["slow"].T).reshape(PPC)
        i0 = c * IPC
        for r_ in range(IPC):
            gi = i0 + r_
            if gi < m:
                slow[gi] = sl[r_ * K_CAP:(r_ + 1) * K_CAP]
    slow = (slow + f32(Smm_b1[0])).astype(f32)
    return (fast + slow).astype(f32)



# revision 4
# speedup vs baseline: 1.1884x; 1.1884x over previous
"""CorefModel TRN2 kernel: 8-core SPMD Bass implementation.

Pipeline (3 device launches + host glue):
  L1: span scores  [C=24576] -- candidate-span axis sharded over 8 cores.
      Decomposed span FFNN: score = w1.relu(A[s] + B[s+w] + Wd[w] + b0
      + (sum_t p_t G[t]) / den), with A/B/G = te @ W0-slices computed once
      per token instead of per span (16x FLOP reduction).
  host: mask invalid, stable argsort, greedy non-crossing NMS -> top_idx;
      span embedding assembly (row gathers + head segment-sums from a
      p*te cumsum) -- pure indexing glue, O(m*D).
  L2: src = emb @ c2f + partial pair matrix (contraction-d sharded over
      cores, host sums; fp32 -- selection-critical), alpha/beta =
      emb @ W0a/b h-slices in bf16 (value path).
  host: visibility mask + top_score add, per-row stable top-k -> ante_idx,
      distance buckets.
  L3: pair FFNN slow score, pairs sharded over cores (38 i-rows x 50 each).
      gamma term via fp8 DoubleRow matmuls on zT = ejT * broadcast(eiT);
      ej gathered bf16 + DMA-transposed; alpha/beta/dist folded into the
      same PSUM accumulation via small bf16 selection matmuls; final
      relu+w1+reduce in one DVE scalar_tensor_tensor with accum_out.

All selection-critical arithmetic (L1 scores, L2 pair matrix) is fp32;
the slow-score value path uses bf16/fp8 (tolerates >1e-2 noise).
"""
import numpy as np
import ml_dtypes
from contextlib import ExitStack

import concourse.bass as bass
import concourse.mybir as mybir
import concourse.tile as tile
from concourse import bacc
from concourse.bass_utils import run_bass_kernel_spmd
from concourse.masks import make_identity

dt = mybir.dt
AF = mybir.ActivationFunctionType
ALU = mybir.AluOpType
PM = mybir.MatmulPerfMode

NCORES = 8
T, E, W, F, H = 1536, 768, 16, 20, 1000
D = 2 * E + F + E              # 2324
DP = 2432                      # D padded to 19*128 (order: start,end,head,width,pad)
KC = DP // 128                 # 19 d-chunks
DP8 = 2560                     # L3 pair-term d padded to 20*128 (for fp8 DoubleRow)
KC8 = DP8 // 128               # 20
TC = T // NCORES               # 192 starts per core
COLS = 256                     # per-core t-window (t0 .. t0+256), te padded to 1600
TPAD = T + 64                  # 1600
SPC = TC * W                   # 3072 spans per core
M_CAP, K_CAP = 300, 50
MP = 304                       # m padded to 8*38
IPC = MP // NCORES             # 38 i-rows per core
PPC = IPC * K_CAP              # 1900 pairs per core
PPCB = 1920                    # padded to 15*128 tile blocks
PCH = PPCB // 128              # 15 pair chunks
S3 = 64.0                      # L3 value-path scale (fp8 range), undone via w1
NEG = np.float32(-1e30)

# se' feature permutation: old order [start(768) end(768) width(20) head(768)],
# new order [start end head width pad(108)]
_PERM = np.concatenate([
    np.arange(0, 2 * E),                   # start, end
    np.arange(2 * E + F, 2 * E + F + E),   # head
    np.arange(2 * E, 2 * E + F),           # width
]).astype(np.int64)            # maps new[0:2324] -> old index

_BIAS_C2F = 2430               # emb slot fixed to 1.0 (c2f bias row)
_BIAS_B0 = 2431                # emb slot fixed to 1.0 (Smm_b0 row via w0b)


def _pad_rows(a, rows):
    out = np.zeros((rows,) + a.shape[1:], a.dtype)
    out[:a.shape[0]] = a
    return out


# ---------------------------------------------------------------- L1 ----
def build_l1():
    nc = bacc.Bacc(trn_type="TRN2", target_bir_lowering=False, debug=False)
    te_loc = nc.dram_tensor("te_loc", [COLS, E], dt.float32, kind="ExternalInput").ap()
    p_loc = nc.dram_tensor("p_loc", [1, COLS], dt.float32, kind="ExternalInput").ap()
    r_loc = nc.dram_tensor("r_loc", [1, SPC], dt.float32, kind="ExternalInput").ap()
    w0abd = nc.dram_tensor("w0abd", [8, 18, 128, H // 8], dt.float32, kind="ExternalInput").ap()
    w0wd = nc.dram_tensor("w0wd", [F + 1, H], dt.float32, kind="ExternalInput").ap()
    we_aug = nc.dram_tensor("we_aug", [F + 1, W], dt.float32, kind="ExternalInput").ap()
    w1_in = nc.dram_tensor("w1_in", [H, 1], dt.float32, kind="ExternalInput").ap()
    score_out = nc.dram_tensor("score", [1, SPC], dt.float32, kind="ExternalOutput").ap()

    HC = 8
    HB = H // HC  # 125
    NT = 512

    with tile.TileContext(nc) as tc, ExitStack() as ctx:
        cst = ctx.enter_context(tc.tile_pool(name="cst", bufs=1))
        ps = ctx.enter_context(tc.tile_pool(name="ps", bufs=2, space="PSUM"))
        scps = ctx.enter_context(tc.tile_pool(name="scps", bufs=1, space="PSUM"))
        npool = ctx.enter_context(tc.tile_pool(name="npool", bufs=3))
        abp = ctx.enter_context(tc.tile_pool(name="abp", bufs=3))
        comb = ctx.enter_context(tc.tile_pool(name="comb", bufs=3))

        # --- constant loads (G-part of W first so PE starts early)
        te_sb = cst.tile([128, 2, E], dt.float32, tag="te_sb")
        nc.sync.dma_start(out=te_sb[:], in_=te_loc.rearrange("(c p) e -> p c e", p=128))
        ident = cst.tile([128, 128], dt.float32, tag="ident")
        make_identity(nc, ident)
        wdsb = cst.tile([F + 1, H], dt.float32, tag="wdsb")
        nc.sync.dma_start(out=wdsb[:], in_=w0wd[:, :])
        wesb = cst.tile([F + 1, W], dt.float32, tag="wesb")
        nc.sync.dma_start(out=wesb[:], in_=we_aug[:, :])
        w1sb = cst.tile([HB, HC], dt.float32, tag="w1sb")
        nc.sync.dma_start(out=w1sb[:], in_=w1_in.rearrange("(c p) one -> p (c one)", p=HB))
        p_bc = cst.tile([128, COLS], dt.float32, tag="p_bc")
        nc.sync.dma_start(out=p_bc[:], in_=p_loc[0:1, :].broadcast_to([128, COLS]))
        r_bc = cst.tile([128, SPC], dt.float32, tag="r_bc")
        nc.sync.dma_start(out=r_bc[:], in_=r_loc[0:1, :].broadcast_to([128, SPC]))

        # --- te^T tiles [128e, 256t] x6
        teT = []
        for ec in range(6):
            t_ = cst.tile([128, COLS], dt.float32, tag=f"teT{ec}", name=f"teT{ec}")
            teT.append(t_)
        for tcki in range(2):
            for ec in range(6):
                pt = ps.tile([128, 128], dt.float32, tag="acc")
                nc.tensor.transpose(out=pt[:], in_=te_sb[:, tcki, ec * 128:(ec + 1) * 128],
                                    identity=ident[:])
                nc.scalar.copy(teT[ec][:, tcki * 128:(tcki + 1) * 128], pt[:])

        sc_ps = [scps.tile([1, NT], dt.float32, tag=f"sc{j}", name=f"sc{j}")
                 for j in range(SPC // NT)]

        wpool = ctx.enter_context(tc.tile_pool(name="wpool", bufs=3))

        def mat_tile(whc, m3, tag):
            t_ = abp.tile([HB, COLS], dt.float32, tag=tag, name=tag)
            acc = ps.tile([HB, COLS], dt.float32, tag="acc", name="acc")
            for ec in range(6):
                nc.tensor.matmul(acc[:],
                                 lhsT=whc[:, m3 * 6 + ec, :],
                                 rhs=teT[ec][:],
                                 start=(ec == 0), stop=(ec == 5))
            nc.scalar.copy(t_[:], acc[:])
            return t_

        # --- per-h-chunk pipeline (W streamed per h-chunk, G chunks first)
        for hc in range(HC):
            whc = wpool.tile([128, 18, HB], dt.float32, tag="whc", name="whc")
            nc.sync.dma_start(out=whc[:], in_=w0abd[hc].rearrange("c p h -> p c h"))
            g_t = mat_tile(whc, 2, "g_t")
            pg = npool.tile([HB, COLS], dt.float32, tag="pg", name="pg")
            nc.vector.tensor_mul(pg[:], g_t[:], p_bc[:HB, :])
            numw = npool.tile([HB, SPC], dt.float32, tag="numw", name="numw")
            n3 = numw[:].rearrange("p (s w) -> p s w", w=W)
            nc.gpsimd.tensor_copy(n3[:, :, 0], pg[:, 0:TC])
            for w_ in range(1, W):
                nc.gpsimd.tensor_add(n3[:, :, w_], n3[:, :, w_ - 1], pg[:, w_:w_ + TC])
            a_t = mat_tile(whc, 0, "a_t")
            b_t = mat_tile(whc, 1, "b_t")
            wd_ps = ps.tile([HB, W], dt.float32, tag="acc", name="wdacc")
            nc.tensor.matmul(wd_ps[:], lhsT=wdsb[:, hc * HB:(hc + 1) * HB],
                             rhs=wesb[:], start=True, stop=True)
            wd_t = abp.tile([HB, W], dt.float32, tag="wd_t", name="wd_t")
            nc.scalar.copy(wd_t[:], wd_ps[:])

            # full-width combine: x = numW*r + A + B  (three wide DVE ops)
            x = comb.tile([HB, SPC], dt.float32, tag="x", name="x")
            nc.vector.tensor_mul(x[:], numw[:], r_bc[:HB, :])
            x3 = x[:].rearrange("p (s w) -> p s w", w=W)
            a_op = a_t[:, 0:TC][:, :, None].broadcast_to([HB, TC, W])
            nc.vector.tensor_add(x3, x3, a_op)
            bt = b_t[:]
            b_op = bass.AP(bt.tensor, bt.offset, [bt.ap[0], [1, TC], [1, W]])
            nc.vector.tensor_add(x3, x3, b_op)
            # wd folded into relu bias, one ACT op per width
            y = comb.tile([HB, SPC], dt.float32, tag="y", name="y")
            y3 = y[:].rearrange("p (s w) -> p s w", w=W)
            for w_ in range(W):
                nc.scalar.activation(y3[:, :, w_], x3[:, :, w_], AF.Relu,
                                     bias=wd_t[:, w_:w_ + 1])
            for j in range(SPC // NT):
                nc.tensor.matmul(sc_ps[j][:], lhsT=w1sb[:, hc:hc + 1],
                                 rhs=y[:, j * NT:(j + 1) * NT],
                                 start=(hc == 0), stop=(hc == HC - 1))

        for j in range(SPC // NT):
            so = comb.tile([1, NT], dt.float32, tag="so", name="so")
            nc.vector.tensor_copy(so[:], sc_ps[j][:])
            nc.sync.dma_start(out=score_out[:, j * NT:(j + 1) * NT], in_=so[:])

    nc.compile()
    return nc


# ---------------------------------------------------------------- L2 ----
def build_l2():
    nc = bacc.Bacc(trn_type="TRN2", target_bir_lowering=False, debug=False)
    embt_in = nc.dram_tensor("embt_in", [DP, M_CAP], dt.float32, kind="ExternalInput").ap()
    esel_in = nc.dram_tensor("esel_in", [128, 3, M_CAP], dt.float32, kind="ExternalInput").ap()
    c2f_c = nc.dram_tensor("c2f_c", [DP, 384], dt.float32, kind="ExternalInput").ap()
    w0a_c = nc.dram_tensor("w0a_c", [DP, 125], dt.bfloat16, kind="ExternalInput").ap()
    w0b_c = nc.dram_tensor("w0b_c", [DP, 125], dt.bfloat16, kind="ExternalInput").ap()

    pair_out = nc.dram_tensor("pair_part", [M_CAP, M_CAP], dt.float32, kind="ExternalOutput").ap()
    a_out = nc.dram_tensor("alpha_part", [M_CAP, 125], dt.bfloat16, kind="ExternalOutput").ap()
    b_out = nc.dram_tensor("beta_part", [M_CAP, 125], dt.bfloat16, kind="ExternalOutput").ap()

    MM = M_CAP
    with tile.TileContext(nc) as tc, ExitStack() as ctx:
        cst = ctx.enter_context(tc.tile_pool(name="cst", bufs=1))
        ps = ctx.enter_context(tc.tile_pool(name="ps", bufs=2, space="PSUM"))

        ident = cst.tile([128, 128], dt.float32, tag="ident")
        make_identity(nc, ident)
        embT = cst.tile([128, KC, MM], dt.float32, tag="embT")
        nc.sync.dma_start(out=embT[:], in_=embt_in.rearrange("(c p) s -> p c s", p=128))
        esel = cst.tile([128, 3, MM], dt.float32, tag="esel")
        nc.sync.dma_start(out=esel[:], in_=esel_in[:, :, :])
        c2f_sb = cst.tile([128, KC, 384], dt.float32, tag="c2f_sb")
        c2f3 = c2f_c.rearrange("(c p) s -> p c s", p=128)
        for ck in range(KC):
            nc.sync.dma_start(out=c2f_sb[:, ck, :], in_=c2f3[:, ck, :])
        w0a_sb = cst.tile([128, KC, 125], dt.bfloat16, tag="w0a_sb")
        nc.sync.dma_start(out=w0a_sb[:], in_=w0a_c.rearrange("(c p) s -> p c s", p=128))
        w0b_sb = cst.tile([128, KC, 125], dt.bfloat16, tag="w0b_sb")
        nc.sync.dma_start(out=w0b_sb[:], in_=w0b_c.rearrange("(c p) s -> p c s", p=128))

        embT16 = cst.tile([128, KC, MM], dt.bfloat16, tag="embT16")
        nc.vector.tensor_copy(embT16[:], embT[:])

        # --- src slots + partial pair
        slotp = ctx.enter_context(tc.tile_pool(name="slotp", bufs=1))
        srcT = []
        for si in range(3):
            t_ = slotp.tile([128, MM], dt.float32, tag=f"srcT{si}", name=f"srcT{si}")
            srcT.append(t_)
        for si in range(3):
            acc = ps.tile([128, MM], dt.float32, tag="acc_ps")
            for kc in range(KC):
                nc.tensor.matmul(acc[:], lhsT=c2f_sb[:, kc, si * 128:(si + 1) * 128],
                                 rhs=embT[:, kc, :], start=(kc == 0), stop=(kc == KC - 1))
            nc.scalar.copy(srcT[si][:], acc[:])
        prow = ctx.enter_context(tc.tile_pool(name="prow", bufs=2))
        for it in range(3):
            inn = min(128, MM - it * 128)
            acc = ps.tile([128, MM], dt.float32, tag="acc_ps")
            for si in range(3):
                nc.tensor.matmul(acc[:inn, :], lhsT=srcT[si][:, it * 128:it * 128 + inn],
                                 rhs=esel[:, si, :], start=(si == 0), stop=(si == 2))
            pr = prow.tile([128, MM], dt.float32, tag="pr")
            nc.scalar.copy(pr[:inn, :], acc[:inn, :])
            nc.sync.dma_start(out=pair_out[it * 128:it * 128 + inn, :], in_=pr[:inn, :])

        # --- alpha/beta h-chunk (125 cols of H per core), bf16 value path
        abrow = ctx.enter_context(tc.tile_pool(name="abrow", bufs=2))
        for wsb_, out_ in ((w0a_sb, a_out), (w0b_sb, b_out)):
            acc = ps.tile([125, MM], dt.float32, tag="acc_ps")
            for kc in range(KC):
                nc.tensor.matmul(acc[:], lhsT=wsb_[:, kc, :], rhs=embT16[:, kc, :],
                                 start=(kc == 0), stop=(kc == KC - 1))
            abT = abrow.tile([125, MM], dt.float32, tag="abT")
            nc.scalar.copy(abT[:], acc[:])
            for g in range(3):
                gn = min(128, MM - g * 128)
                pt = ps.tile([128, 125], dt.float32, tag="tr_ps")
                nc.tensor.transpose(out=pt[:gn, :], in_=abT[:, g * 128:g * 128 + gn],
                                    identity=ident[:125, :125])
                ab_r = abrow.tile([128, 125], dt.bfloat16, tag="ab_r")
                nc.scalar.copy(ab_r[:gn, :], pt[:gn, :])
                nc.sync.dma_start(out=out_[g * 128:g * 128 + gn, :], in_=ab_r[:gn, :])

    nc.compile()
    return nc


# ---------------------------------------------------------------- L3 ----
def build_l3():
    nc = bacc.Bacc(trn_type="TRN2", target_bir_lowering=False, debug=False)
    embb = nc.dram_tensor("embb", [MP, DP8], dt.bfloat16, kind="ExternalInput").ap()
    jidx = nc.dram_tensor("jidx", [PPCB, 1], dt.int32, kind="ExternalInput").ap()
    eit_in = nc.dram_tensor("eit_in", [128, KC8, IPC], dt.bfloat16, kind="ExternalInput").ap()
    ind_in = nc.dram_tensor("ind_in", [IPC, PPCB], dt.bfloat16, kind="ExternalInput").ap()
    jind_in = nc.dram_tensor("jind_in", [128, 3, PPCB], dt.bfloat16, kind="ExternalInput").ap()
    aloc_in = nc.dram_tensor("aloc_in", [IPC, H], dt.bfloat16, kind="ExternalInput").ap()
    beta_in = nc.dram_tensor("beta_in", [128, 3, H], dt.bfloat16, kind="ExternalInput").ap()
    demb_in = nc.dram_tensor("demb_in", [F, PPCB], dt.bfloat16, kind="ExternalInput").ap()
    w0d_in = nc.dram_tensor("w0d_in", [F, H], dt.bfloat16, kind="ExternalInput").ap()
    w0c_in = nc.dram_tensor("w0c_in", [128, KC8, H], dt.float8e4, kind="ExternalInput").ap()
    w1row = nc.dram_tensor("w1row", [1, H], dt.float32, kind="ExternalInput").ap()
    slow_out = nc.dram_tensor("slow", [128, PCH], dt.float32, kind="ExternalOutput").ap()

    with tile.TileContext(nc) as tc, ExitStack() as ctx:
        cst = ctx.enter_context(tc.tile_pool(name="cst", bufs=1))
        w0c_sb = cst.tile([128, KC8, H], dt.float8e4, tag="w0c_sb")
        nc.sync.dma_start(out=w0c_sb[:], in_=w0c_in[:, :, :])
        eit_sb = cst.tile([128, KC8, IPC], dt.bfloat16, tag="eit_sb")
        nc.sync.dma_start(out=eit_sb[:], in_=eit_in[:, :, :])
        ind_sb = cst.tile([IPC, PPCB], dt.bfloat16, tag="ind_sb")
        nc.sync.dma_start(out=ind_sb[:], in_=ind_in[:, :])
        jind_sb = cst.tile([128, 3, PPCB], dt.bfloat16, tag="jind_sb")
        nc.sync.dma_start(out=jind_sb[:], in_=jind_in[:, :, :])
        aloc_sb = cst.tile([IPC, H], dt.bfloat16, tag="aloc_sb")
        nc.sync.dma_start(out=aloc_sb[:], in_=aloc_in[:, :])
        beta_sb = cst.tile([128, 3, H], dt.bfloat16, tag="beta_sb")
        nc.sync.dma_start(out=beta_sb[:], in_=beta_in[:, :, :])
        demb_sb = cst.tile([F, PPCB], dt.bfloat16, tag="demb_sb")
        nc.sync.dma_start(out=demb_sb[:], in_=demb_in[:, :])
        w0d_sb = cst.tile([F, H], dt.bfloat16, tag="w0d_sb")
        nc.sync.dma_start(out=w0d_sb[:], in_=w0d_in[:, :])
        w1_bc = cst.tile([128, H], dt.float32, tag="w1_bc")
        nc.sync.dma_start(out=w1_bc[:], in_=w1row[0:1, :].broadcast_to([128, H]))
        jidx_sb = cst.tile([128, PCH], dt.int32, tag="jidx_sb")
        nc.sync.dma_start(out=jidx_sb[:], in_=jidx.rearrange("(c p) one -> p (c one)", p=128))
        slow_sb = cst.tile([128, PCH], dt.float32, tag="slow_sb")

        ejT = cst.tile([128, KC8, PPCB], dt.bfloat16, tag="ejT")
        zT = cst.tile([128, KC8, PPCB], dt.float8e4, tag="zT")
        # pad pair columns (no eiT data there): zero once
        nc.vector.memset(zT[:, :, PPC:PPCB], 0.0)

        wk = ctx.enter_context(tc.tile_pool(name="wk", bufs=3))
        ps = ctx.enter_context(tc.tile_pool(name="ps", bufs=2, space="PSUM"))

        # gather + transpose ej rows
        for pc in range(PCH):
            ej = wk.tile([128, DP8], dt.bfloat16, tag="ej", name="ej")
            nc.gpsimd.indirect_dma_start(
                out=ej[:], out_offset=None, in_=embb[:, :],
                in_offset=bass.IndirectOffsetOnAxis(ap=jidx_sb[:, pc:pc + 1], axis=0))
            nc.sync.dma_start_transpose(ejT[:, :, pc * 128:(pc + 1) * 128], ej[:])

        # zT = ejT * broadcast(eiT), in group-aligned spans of 500 pairs
        ZCH = 500  # 10 i-groups of 50
        zlims = list(range(0, PPC, ZCH)) + [PPC]
        for kc in range(KC8):
            for zi in range(len(zlims) - 1):
                a, b = zlims[zi], zlims[zi + 1]
                ia, ib = a // K_CAP, b // K_CAP
                o3 = zT[:, kc, a:b].rearrange("p (i k) -> p i k", k=K_CAP)
                e3 = ejT[:, kc, a:b].rearrange("p (i k) -> p i k", k=K_CAP)
                w3 = eit_sb[:, kc, ia:ib][:, :, None].broadcast_to(
                    [128, ib - ia, K_CAP])
                nc.vector.tensor_mul(o3, e3, w3)

        # per pair-block: delta + alpha + beta + gamma in one PSUM group
        # (H split in 500-wide halves -- matmul output must fit one PSUM
        # bank), then relu * w1 + reduce via stt accum_out
        NH = H // 2
        half_acc = [cst.tile([128, PCH], dt.float32, tag=f"hacc{hh}", name=f"hacc{hh}")
                    for hh in range(2)]
        for pc in range(PCH):
            blk = slice(pc * 128, (pc + 1) * 128)
            for hh in range(2):
                hs = slice(hh * NH, (hh + 1) * NH)
                acc = ps.tile([128, NH], dt.float32, tag="acc", name="acc")
                nc.tensor.matmul(acc[:], lhsT=demb_sb[:, blk], rhs=w0d_sb[:, hs],
                                 start=True, stop=False, skip_group_check=True)
                nc.tensor.matmul(acc[:], lhsT=ind_sb[:, blk], rhs=aloc_sb[:, hs],
                                 start=False, stop=False, skip_group_check=True)
                for g in range(3):
                    nc.tensor.matmul(acc[:], lhsT=jind_sb[:, g, blk],
                                     rhs=beta_sb[:, g, hs],
                                     start=False, stop=False, skip_group_check=True)
                for g in range(KC8 // 2):
                    nc.tensor.matmul(acc[:], lhsT=zT[:, 2 * g:2 * g + 2, blk],
                                     rhs=w0c_sb[:, 2 * g:2 * g + 2, hs],
                                     perf_mode=PM.DoubleRow,
                                     start=False, stop=(g == KC8 // 2 - 1),
                                     skip_group_check=True)
                y = wk.tile([128, NH], dt.bfloat16, tag="y", name="y")
                nc.vector.scalar_tensor_tensor(
                    y[:], acc[:], 0.0, w1_bc[:, hs],
                    ALU.max, ALU.mult, accum_out=half_acc[hh][:, pc:pc + 1])
        nc.vector.tensor_add(slow_sb[:], half_acc[0][:], half_acc[1][:])
        nc.sync.dma_start(out=slow_out[:, :], in_=slow_sb[:])

    nc.compile()
    return nc


# ------------------------------------------------------------- host -----
_BUILT = {}
TRACE = False
PERF = {}


def _run(name, nc, in_maps, cores):
    try:
        res = run_bass_kernel_spmd(nc, in_maps, core_ids=cores, trace=TRACE)
    except ModuleNotFoundError:
        res = run_bass_kernel_spmd(nc, in_maps, core_ids=cores)
    if res.exec_time_ns is not None:
        PERF[name] = res.exec_time_ns
    return res


def _get(name, builder):
    if name not in _BUILT:
        _BUILT[name] = builder()
    return _BUILT[name]


def _bucket_dist(d):
    df = d.astype(np.float32)
    with np.errstate(divide="ignore"):
        log_idx = np.floor(np.log(np.maximum(df, np.float32(1.0))) /
                           np.float32(np.log(2.0))) + np.float32(3.0)
    comb = np.where(d <= 4, df, log_idx)
    return np.clip(comb, 0, 9).astype(np.int32)


def _nms(order, starts, ends, m):
    top = []
    ts = np.zeros(m, np.int64)
    te_ = np.zeros(m, np.int64)
    n = 0
    i = 0
    C = order.shape[0]
    while n < m and i < C:
        idx = order[i]
        s, e = starts[idx], ends[idx]
        if n:
            a = ts[:n]
            b = te_[:n]
            cross = (((s < a) & (e < b) & (e >= a)) | ((s > a) & (s <= b) & (e > b))).any()
        else:
            cross = False
        if not cross:
            top.append(idx)
            ts[n] = s
            te_[n] = e
            n += 1
        i += 1
    return np.asarray(top, np.int64)


def kernel(tokens_embed, Wh_w, Wh_b, width_emb, dist_emb,
           Sm_w0, Sm_b0, Sm_w1, Sm_b1,
           Smm_w0, Smm_b0, Smm_w1, Smm_b1,
           c2f_w, c2f_b, m, k):
    m = int(m)
    k = int(k)
    assert m == M_CAP and k == K_CAP, (m, k)
    f32 = np.float32
    bf16 = ml_dtypes.bfloat16
    f8 = ml_dtypes.float8_e4m3fn
    te = np.ascontiguousarray(tokens_embed, f32)
    cores = list(range(NCORES))

    # ---- host prep (index/pad/exp glue)
    te_pad = np.concatenate([te, np.repeat(te[-1:], TPAD - T, 0)], 0)
    tok = (te @ Wh_w.astype(f32))[:, 0] + f32(Wh_b[0])
    p_full = np.exp(tok.astype(f32)).astype(f32)
    p_pad = np.concatenate([p_full, np.zeros(TPAD - T, f32)])
    # den/r per (s, w): sequential fp32 prefix
    den = np.empty((T, W), f32)
    acc = p_full.copy()
    den[:, 0] = acc
    for w_ in range(1, W):
        nxt = np.concatenate([p_full[w_:], np.zeros(w_, f32)])
        acc = (acc + nxt).astype(f32)
        den[:, w_] = acc
    r_full = (f32(1.0) / den).astype(f32)
    # cumulative p*te for the head segment sums: cum[t] = sum_{t'<t} p*te
    cum_pte = np.zeros((T + 1, E), f32)
    np.cumsum((p_full[:, None] * te).astype(f32), axis=0, dtype=f32,
              out=cum_pte[1:])

    # L1 inputs
    w0abd_flat = np.concatenate([Sm_w0[0:E], Sm_w0[E:2 * E], Sm_w0[2 * E + F:]], 0).astype(f32)
    w0abd = np.ascontiguousarray(
        w0abd_flat.reshape(18, 128, 8, 125).transpose(2, 0, 1, 3))
    w0wd = np.concatenate([Sm_w0[2 * E:2 * E + F], Sm_b0[None, :]], 0).astype(f32)
    we_aug = np.concatenate([width_emb.T, np.ones((1, W))], 0).astype(f32)
    in1 = []
    for c in cores:
        t0 = c * TC
        in1.append(dict(
            te_loc=np.ascontiguousarray(te_pad[t0:t0 + COLS]),
            p_loc=np.ascontiguousarray(p_pad[None, t0:t0 + COLS]),
            r_loc=np.ascontiguousarray(r_full[t0:t0 + TC].reshape(1, SPC)),
            w0abd=w0abd, w0wd=w0wd, we_aug=we_aug,
            w1_in=Sm_w1.astype(f32)))
    nc1 = _get("l1", build_l1)
    res1 = _run("l1", nc1, in1, cores)
    scores = np.concatenate([res1.results[c]["score"][0] for c in cores])

    # ---- host: mask + NMS
    starts = np.repeat(np.arange(T, dtype=np.int64), W)
    widths = np.tile(np.arange(W, dtype=np.int64), T)
    ends = starts + widths
    valid = ends < T
    sc = np.where(valid, (scores + f32(Sm_b1[0])).astype(f32), NEG).astype(f32)
    order = np.argsort(-sc, kind="stable")
    top_idx = _nms(order, starts, ends, m)
    ts_, tw = starts[top_idx], widths[top_idx]
    tec = np.minimum(ts_ + tw, T - 1)
    top_score = sc[top_idx]

    # ---- host: span embedding assembly (indexing glue)
    head = ((cum_pte[tec + 1] - cum_pte[ts_]) *
            r_full[ts_, tw][:, None]).astype(f32)
    emb = np.zeros((M_CAP, DP), f32)
    emb[:, 0:E] = te[ts_]
    emb[:, E:2 * E] = te[tec]
    emb[:, 2 * E:3 * E] = head
    emb[:, 3 * E:3 * E + F] = width_emb.astype(f32)[tw]
    emb[:, _BIAS_C2F] = 1.0
    emb[:, _BIAS_B0] = 1.0
    embt_in = np.ascontiguousarray(emb.T)

    # ---- L2
    perm = _PERM
    c2f_pad = np.zeros((DP, DP), f32)
    c2f_pad[:D, :D] = c2f_w.astype(f32)[np.ix_(perm, perm)]
    c2f_pad[_BIAS_C2F, :D] = c2f_b.astype(f32)[perm]
    w0a_pad = np.zeros((DP, H), f32)
    w0a_pad[:D] = Smm_w0[0:D].astype(f32)[perm] * f32(S3)
    w0b_pad = np.zeros((DP, H), f32)
    w0b_pad[:D] = Smm_w0[D:2 * D].astype(f32)[perm] * f32(S3)
    w0b_pad[_BIAS_B0] = Smm_b0.astype(f32) * f32(S3)
    in2 = []
    for c in cores:
        hc0 = c * 125
        cs = [3 * c, 3 * c + 1, 3 * c + 2]
        c2fc = np.zeros((DP, 384), f32)
        eselc = np.zeros((128, 3, M_CAP), f32)
        for si, ch in enumerate(cs):
            if ch < KC:
                c2fc[:, si * 128:(si + 1) * 128] = c2f_pad[:, ch * 128:(ch + 1) * 128]
                eselc[:, si, :] = embt_in[ch * 128:(ch + 1) * 128, :]
        in2.append(dict(
            embt_in=embt_in, esel_in=eselc, c2f_c=c2fc,
            w0a_c=np.ascontiguousarray(w0a_pad[:, hc0:hc0 + 125]).astype(bf16),
            w0b_c=np.ascontiguousarray(w0b_pad[:, hc0:hc0 + 125]).astype(bf16)))
    nc2 = _get("l2", build_l2)
    res2 = _run("l2", nc2, in2, cores)
    pair = np.zeros((m, m), f32)
    for c in cores:
        pair += res2.results[c]["pair_part"]
    pair = pair.astype(f32)
    alpha16 = np.concatenate([res2.results[c]["alpha_part"] for c in cores], 1)
    beta16 = np.concatenate([res2.results[c]["beta_part"] for c in cores], 1)

    # ---- host: visibility + top-k
    offset = np.arange(m, dtype=np.int64)[:, None] - np.arange(m, dtype=np.int64)[None, :]
    vis = offset >= 1
    all_score = (np.where(vis, f32(0.0), NEG).astype(f32) + top_score[:, None]).astype(f32)
    all_score = (all_score + pair).astype(f32)
    ante_idx = np.argsort(-all_score, axis=1, kind="stable")[:, :k]
    fast = np.take_along_axis(all_score, ante_idx, axis=1).astype(f32)
    ante_off = np.take_along_axis(offset, ante_idx, axis=1)
    dbuck = _bucket_dist(ante_off)

    # ---- L3
    emb16 = np.zeros((MP, DP8), bf16)
    emb16[:m, :DP] = emb.astype(bf16)
    emb16_f = emb16.astype(f32)
    w0c_pad = np.zeros((DP8, H), f32)
    w0c_pad[:D] = Smm_w0[2 * D:3 * D].astype(f32)[perm] * f32(S3)
    w0c8 = np.ascontiguousarray(
        w0c_pad.reshape(KC8, 128, H).transpose(1, 0, 2)).astype(f8)
    w0d16 = (Smm_w0[3 * D:3 * D + F].astype(f32) * f32(S3)).astype(bf16)
    ind16 = np.zeros((IPC, PPCB), bf16)
    for r_ in range(IPC):
        ind16[r_, r_ * K_CAP:(r_ + 1) * K_CAP] = 1.0
    beta_pad = _pad_rows(np.asarray(beta16, bf16), 384)
    beta_c = np.ascontiguousarray(beta_pad.reshape(3, 128, H).transpose(1, 0, 2))
    alpha_pad = _pad_rows(np.asarray(alpha16, bf16), MP)
    in3 = []
    for c in cores:
        i0 = c * IPC
        jj = np.zeros(PPCB, np.int32)
        dmb = np.zeros((F, PPCB), f32)
        for r_ in range(IPC):
            gi = i0 + r_
            if gi < m:
                sl = slice(r_ * K_CAP, (r_ + 1) * K_CAP)
                jj[sl] = ante_idx[gi]
                dmb[:, sl] = dist_emb.astype(f32)[dbuck[gi]].T
        eit = np.ascontiguousarray(
            emb16_f[i0:i0 + IPC].reshape(IPC, KC8, 128).transpose(2, 1, 0)).astype(bf16)
        jind = np.zeros((128, 3, PPCB), bf16)
        jind[jj % 128, jj // 128, np.arange(PPCB)] = 1.0
        in3.append(dict(
            embb=emb16, jidx=jj[:, None], eit_in=eit, ind_in=ind16,
            jind_in=jind, aloc_in=np.ascontiguousarray(alpha_pad[i0:i0 + IPC]),
            beta_in=beta_c, demb_in=dmb.astype(bf16), w0d_in=w0d16,
            w0c_in=w0c8, w1row=np.ascontiguousarray(Smm_w1.astype(f32).T / f32(S3))))
    nc3 = _get("l3", build_l3)
    res3 = _run("l3", nc3, in3, cores)
    slow = np.zeros((m, k), f32)
    for c in cores:
        sl = np.ascontiguousarray(res3.results[c]["slow"].T).reshape(PPCB)
        i0 = c * IPC
        for r_ in range(IPC):
            gi = i0 + r_
            if gi < m:
                slow[gi] = sl[r_ * K_CAP:(r_ + 1) * K_CAP]
    slow = (slow + f32(Smm_b1[0])).astype(f32)
    return (fast + slow).astype(f32)


# revision 8
# speedup vs baseline: 1.2579x; 1.0585x over previous
"""CorefModel TRN2 kernel: 8-core SPMD Bass implementation.

Pipeline (3 device launches + host glue):
  L1: span scores  [C=24576] -- candidate-span axis sharded over 8 cores.
      Decomposed span FFNN: score = w1.relu(A[s] + B[s+w] + Wd[w] + b0
      + (sum_t p_t G[t]) / den), with A/B/G = te @ W0-slices computed once
      per token instead of per span (16x FLOP reduction).
  host: mask invalid, stable argsort, greedy non-crossing NMS -> top_idx;
      span embedding assembly (row gathers + head segment-sums from a
      p*te cumsum) -- pure indexing glue, O(m*D).
  L2: src = emb @ c2f + partial pair matrix (contraction-d sharded over
      cores, host sums; fp32 -- selection-critical), alpha/beta =
      emb @ W0a/b h-slices in bf16 (value path).
  host: visibility mask + top_score add, per-row stable top-k -> ante_idx,
      distance buckets.
  L3: pair FFNN slow score, pairs sharded over cores (38 i-rows x 50 each).
      gamma term via fp8 DoubleRow matmuls on zT = ejT * broadcast(eiT);
      ej gathered bf16 + DMA-transposed; alpha/beta/dist folded into the
      same PSUM accumulation via small bf16 selection matmuls; final
      relu+w1+reduce in one DVE scalar_tensor_tensor with accum_out.

All selection-critical arithmetic (L1 scores, L2 pair matrix) is fp32;
the slow-score value path uses bf16/fp8 (tolerates >1e-2 noise).
"""
import numpy as np
import ml_dtypes
from contextlib import ExitStack

import concourse.bass as bass
import concourse.mybir as mybir
import concourse.tile as tile
from concourse import bacc
from concourse.bass_utils import run_bass_kernel_spmd
from concourse.masks import make_identity

dt = mybir.dt
AF = mybir.ActivationFunctionType
ALU = mybir.AluOpType
PM = mybir.MatmulPerfMode

NCORES = 8
T, E, W, F, H = 1536, 768, 16, 20, 1000
D = 2 * E + F + E              # 2324
DP = 2432                      # D padded to 19*128 (order: start,end,head,width,pad)
KC = DP // 128                 # 19 d-chunks
DP8 = 2560                     # L3 pair-term d padded to 20*128 (for fp8 DoubleRow)
KC8 = DP8 // 128               # 20
TC = T // NCORES               # 192 starts per core
COLS = 256                     # per-core t-window (t0 .. t0+256), te padded to 1600
TPAD = T + 64                  # 1600
SPC = TC * W                   # 3072 spans per core
M_CAP, K_CAP = 300, 50
MP = 304                       # m padded to 8*38
IPC = MP // NCORES             # 38 i-rows per core
PPC = IPC * K_CAP              # 1900 pairs per core
PPCB = 1920                    # padded to 15*128 tile blocks
PCH = PPCB // 128              # 15 pair chunks
S3 = 64.0                      # L3 value-path scale (fp8 range), undone via w1
NEG = np.float32(-1e30)

# se' feature permutation: old order [start(768) end(768) width(20) head(768)],
# new order [start end head width pad(108)]
_PERM = np.concatenate([
    np.arange(0, 2 * E),                   # start, end
    np.arange(2 * E + F, 2 * E + F + E),   # head
    np.arange(2 * E, 2 * E + F),           # width
]).astype(np.int64)            # maps new[0:2324] -> old index

_BIAS_C2F = 2430               # emb slot fixed to 1.0 (c2f bias row)
_BIAS_B0 = 2431                # emb slot fixed to 1.0 (Smm_b0 row via w0b)


def _pad_rows(a, rows):
    out = np.zeros((rows,) + a.shape[1:], a.dtype)
    out[:a.shape[0]] = a
    return out


# ---------------------------------------------------------------- L1 ----
def build_l1():
    nc = bacc.Bacc(trn_type="TRN2", target_bir_lowering=False, debug=False)
    te_loc = nc.dram_tensor("te_loc", [COLS, E], dt.float32, kind="ExternalInput").ap()
    p_loc = nc.dram_tensor("p_loc", [1, COLS], dt.float32, kind="ExternalInput").ap()
    r_loc = nc.dram_tensor("r_loc", [1, SPC], dt.float32, kind="ExternalInput").ap()
    w0abd = nc.dram_tensor("w0abd", [8, 18, 128, H // 8], dt.float32, kind="ExternalInput").ap()
    w0wd = nc.dram_tensor("w0wd", [F + 1, H], dt.float32, kind="ExternalInput").ap()
    we_aug = nc.dram_tensor("we_aug", [F + 1, W], dt.float32, kind="ExternalInput").ap()
    w1_in = nc.dram_tensor("w1_in", [H, 1], dt.float32, kind="ExternalInput").ap()
    score_out = nc.dram_tensor("score", [1, SPC], dt.float32, kind="ExternalOutput").ap()

    HC = 8
    HB = H // HC  # 125
    NT = 512

    with tile.TileContext(nc) as tc, ExitStack() as ctx:
        cst = ctx.enter_context(tc.tile_pool(name="cst", bufs=1))
        ps = ctx.enter_context(tc.tile_pool(name="ps", bufs=2, space="PSUM"))
        scps = ctx.enter_context(tc.tile_pool(name="scps", bufs=1, space="PSUM"))
        npool = ctx.enter_context(tc.tile_pool(name="npool", bufs=3))
        abp = ctx.enter_context(tc.tile_pool(name="abp", bufs=3))
        comb = ctx.enter_context(tc.tile_pool(name="comb", bufs=3))

        # --- constant loads (G-part of W first so PE starts early)
        te_sb = cst.tile([128, 2, E], dt.float32, tag="te_sb")
        nc.sync.dma_start(out=te_sb[:], in_=te_loc.rearrange("(c p) e -> p c e", p=128))
        ident = cst.tile([128, 128], dt.float32, tag="ident")
        make_identity(nc, ident)
        wdsb = cst.tile([F + 1, H], dt.float32, tag="wdsb")
        nc.sync.dma_start(out=wdsb[:], in_=w0wd[:, :])
        wesb = cst.tile([F + 1, W], dt.float32, tag="wesb")
        nc.sync.dma_start(out=wesb[:], in_=we_aug[:, :])
        w1sb = cst.tile([HB, HC], dt.float32, tag="w1sb")
        nc.sync.dma_start(out=w1sb[:], in_=w1_in.rearrange("(c p) one -> p (c one)", p=HB))
        p_bc = cst.tile([128, COLS], dt.float32, tag="p_bc")
        nc.sync.dma_start(out=p_bc[:], in_=p_loc[0:1, :].broadcast_to([128, COLS]))
        r_bc = cst.tile([128, SPC], dt.float32, tag="r_bc")
        nc.sync.dma_start(out=r_bc[:], in_=r_loc[0:1, :].broadcast_to([128, SPC]))

        # --- te^T tiles [128e, 256t] x6
        teT = []
        for ec in range(6):
            t_ = cst.tile([128, COLS], dt.float32, tag=f"teT{ec}", name=f"teT{ec}")
            teT.append(t_)
        for tcki in range(2):
            for ec in range(6):
                pt = ps.tile([128, 128], dt.float32, tag="acc")
                nc.tensor.transpose(out=pt[:], in_=te_sb[:, tcki, ec * 128:(ec + 1) * 128],
                                    identity=ident[:])
                nc.scalar.copy(teT[ec][:, tcki * 128:(tcki + 1) * 128], pt[:])

        sc_ps = [scps.tile([1, NT], dt.float32, tag=f"sc{j}", name=f"sc{j}")
                 for j in range(SPC // NT)]

        wpool = ctx.enter_context(tc.tile_pool(name="wpool", bufs=3))

        def mat_tile(whc, m3, tag):
            t_ = abp.tile([HB, COLS], dt.float32, tag=tag, name=tag)
            acc = ps.tile([HB, COLS], dt.float32, tag="acc", name="acc")
            for ec in range(6):
                nc.tensor.matmul(acc[:],
                                 lhsT=whc[:, m3 * 6 + ec, :],
                                 rhs=teT[ec][:],
                                 start=(ec == 0), stop=(ec == 5))
            nc.scalar.copy(t_[:], acc[:])
            return t_

        # --- per-h-chunk pipeline (W streamed per h-chunk, G chunks first)
        for hc in range(HC):
            whc = wpool.tile([128, 18, HB], dt.float32, tag="whc", name="whc")
            nc.sync.dma_start(out=whc[:], in_=w0abd[hc].rearrange("c p h -> p c h"))
            g_t = mat_tile(whc, 2, "g_t")
            pg = npool.tile([HB, COLS], dt.float32, tag="pg", name="pg")
            nc.vector.tensor_mul(pg[:], g_t[:], p_bc[:HB, :])
            numw = npool.tile([HB, SPC], dt.float32, tag="numw", name="numw")
            n3 = numw[:].rearrange("p (s w) -> p s w", w=W)
            nc.gpsimd.tensor_copy(n3[:, :, 0], pg[:, 0:TC])
            for w_ in range(1, W):
                nc.gpsimd.tensor_add(n3[:, :, w_], n3[:, :, w_ - 1], pg[:, w_:w_ + TC])
            a_t = mat_tile(whc, 0, "a_t")
            b_t = mat_tile(whc, 1, "b_t")
            wd_ps = ps.tile([HB, W], dt.float32, tag="acc", name="wdacc")
            nc.tensor.matmul(wd_ps[:], lhsT=wdsb[:, hc * HB:(hc + 1) * HB],
                             rhs=wesb[:], start=True, stop=True)
            wd_t = abp.tile([HB, W], dt.float32, tag="wd_t", name="wd_t")
            nc.scalar.copy(wd_t[:], wd_ps[:])

            # full-width combine: x = numW*r + A + B  (three wide DVE ops)
            x = comb.tile([HB, SPC], dt.float32, tag="x", name="x")
            nc.vector.tensor_mul(x[:], numw[:], r_bc[:HB, :])
            x3 = x[:].rearrange("p (s w) -> p s w", w=W)
            a_op = a_t[:, 0:TC][:, :, None].broadcast_to([HB, TC, W])
            nc.vector.tensor_add(x3, x3, a_op)
            bt = b_t[:]
            b_op = bass.AP(bt.tensor, bt.offset, [bt.ap[0], [1, TC], [1, W]])
            nc.vector.tensor_add(x3, x3, b_op)
            # wd folded into relu bias, one ACT op per width
            y = comb.tile([HB, SPC], dt.float32, tag="y", name="y")
            y3 = y[:].rearrange("p (s w) -> p s w", w=W)
            for w_ in range(W):
                nc.scalar.activation(y3[:, :, w_], x3[:, :, w_], AF.Relu,
                                     bias=wd_t[:, w_:w_ + 1])
            for j in range(SPC // NT):
                nc.tensor.matmul(sc_ps[j][:], lhsT=w1sb[:, hc:hc + 1],
                                 rhs=y[:, j * NT:(j + 1) * NT],
                                 start=(hc == 0), stop=(hc == HC - 1))

        for j in range(SPC // NT):
            so = comb.tile([1, NT], dt.float32, tag="so", name="so")
            nc.vector.tensor_copy(so[:], sc_ps[j][:])
            nc.sync.dma_start(out=score_out[:, j * NT:(j + 1) * NT], in_=so[:])

    nc.compile()
    return nc


# ---------------------------------------------------------------- L2 ----
def build_l2():
    nc = bacc.Bacc(trn_type="TRN2", target_bir_lowering=False, debug=False)
    embt_in = nc.dram_tensor("embt_in", [DP, M_CAP], dt.float32, kind="ExternalInput").ap()
    esel_in = nc.dram_tensor("esel_in", [128, 3, M_CAP], dt.float32, kind="ExternalInput").ap()
    c2f_c = nc.dram_tensor("c2f_c", [DP, 384], dt.float32, kind="ExternalInput").ap()
    w0a_c = nc.dram_tensor("w0a_c", [DP, 125], dt.bfloat16, kind="ExternalInput").ap()
    w0b_c = nc.dram_tensor("w0b_c", [DP, 125], dt.bfloat16, kind="ExternalInput").ap()

    pair_out = nc.dram_tensor("pair_part", [M_CAP, M_CAP], dt.float32, kind="ExternalOutput").ap()
    a_out = nc.dram_tensor("alpha_part", [M_CAP, 125], dt.bfloat16, kind="ExternalOutput").ap()
    b_out = nc.dram_tensor("beta_part", [M_CAP, 125], dt.bfloat16, kind="ExternalOutput").ap()

    MM = M_CAP
    with tile.TileContext(nc) as tc, ExitStack() as ctx:
        cst = ctx.enter_context(tc.tile_pool(name="cst", bufs=1))
        ps = ctx.enter_context(tc.tile_pool(name="ps", bufs=2, space="PSUM"))

        ident = cst.tile([128, 128], dt.float32, tag="ident")
        make_identity(nc, ident)
        embT = cst.tile([128, KC, MM], dt.float32, tag="embT")
        embT16 = cst.tile([128, KC, MM], dt.bfloat16, tag="embT16")
        c2f_sb = cst.tile([128, KC, 384], dt.float32, tag="c2f_sb")
        embt3 = embt_in.rearrange("(c p) s -> p c s", p=128)
        c2f3 = c2f_c.rearrange("(c p) s -> p c s", p=128)
        # chunked, queue-split loads so the src matmul chain starts on the
        # first chunk instead of a whole-tile barrier
        for ck in range(KC):
            nc.scalar.dma_start(out=embT[:, ck, :], in_=embt3[:, ck, :])
            nc.sync.dma_start(out=c2f_sb[:, ck, :], in_=c2f3[:, ck, :])
            nc.vector.tensor_copy(embT16[:, ck, :], embT[:, ck, :])
        esel = cst.tile([128, 3, MM], dt.float32, tag="esel")
        nc.scalar.dma_start(out=esel[:], in_=esel_in[:, :, :])
        w0a_sb = cst.tile([128, KC, 125], dt.bfloat16, tag="w0a_sb")
        nc.sync.dma_start(out=w0a_sb[:], in_=w0a_c.rearrange("(c p) s -> p c s", p=128))
        w0b_sb = cst.tile([128, KC, 125], dt.bfloat16, tag="w0b_sb")
        nc.sync.dma_start(out=w0b_sb[:], in_=w0b_c.rearrange("(c p) s -> p c s", p=128))

        # --- src slots + partial pair
        slotp = ctx.enter_context(tc.tile_pool(name="slotp", bufs=1))
        srcT = []
        for si in range(3):
            t_ = slotp.tile([128, MM], dt.float32, tag=f"srcT{si}", name=f"srcT{si}")
            srcT.append(t_)
        for si in range(3):
            acc = ps.tile([128, MM], dt.float32, tag="acc_ps")
            for kc in range(KC):
                nc.tensor.matmul(acc[:], lhsT=c2f_sb[:, kc, si * 128:(si + 1) * 128],
                                 rhs=embT[:, kc, :], start=(kc == 0), stop=(kc == KC - 1))
            nc.scalar.copy(srcT[si][:], acc[:])
        prow = ctx.enter_context(tc.tile_pool(name="prow", bufs=2))
        for it in range(3):
            inn = min(128, MM - it * 128)
            acc = ps.tile([128, MM], dt.float32, tag="acc_ps")
            for si in range(3):
                nc.tensor.matmul(acc[:inn, :], lhsT=srcT[si][:, it * 128:it * 128 + inn],
                                 rhs=esel[:, si, :], start=(si == 0), stop=(si == 2))
            pr = prow.tile([128, MM], dt.float32, tag="pr")
            nc.scalar.copy(pr[:inn, :], acc[:inn, :])
            nc.sync.dma_start(out=pair_out[it * 128:it * 128 + inn, :], in_=pr[:inn, :])

        # --- alpha/beta h-chunk (125 cols of H per core), bf16 value path
        abrow = ctx.enter_context(tc.tile_pool(name="abrow", bufs=2))
        for wsb_, out_ in ((w0a_sb, a_out), (w0b_sb, b_out)):
            acc = ps.tile([125, MM], dt.float32, tag="acc_ps")
            for kc in range(KC):
                nc.tensor.matmul(acc[:], lhsT=wsb_[:, kc, :], rhs=embT16[:, kc, :],
                                 start=(kc == 0), stop=(kc == KC - 1))
            abT = abrow.tile([125, MM], dt.float32, tag="abT")
            nc.scalar.copy(abT[:], acc[:])
            for g in range(3):
                gn = min(128, MM - g * 128)
                pt = ps.tile([128, 125], dt.float32, tag="tr_ps")
                nc.tensor.transpose(out=pt[:gn, :], in_=abT[:, g * 128:g * 128 + gn],
                                    identity=ident[:125, :125])
                ab_r = abrow.tile([128, 125], dt.bfloat16, tag="ab_r")
                nc.scalar.copy(ab_r[:gn, :], pt[:gn, :])
                nc.sync.dma_start(out=out_[g * 128:g * 128 + gn, :], in_=ab_r[:gn, :])

    nc.compile()
    return nc


# ---------------------------------------------------------------- L3 ----
def build_l3():
    nc = bacc.Bacc(trn_type="TRN2", target_bir_lowering=False, debug=False)
    embb = nc.dram_tensor("embb", [MP, DP8], dt.bfloat16, kind="ExternalInput").ap()
    jidx = nc.dram_tensor("jidx", [PPCB, 1], dt.int32, kind="ExternalInput").ap()
    eit_in = nc.dram_tensor("eit_in", [128, KC8, IPC], dt.bfloat16, kind="ExternalInput").ap()
    jind_in = nc.dram_tensor("jind_in", [128, 3, PPCB], dt.bfloat16, kind="ExternalInput").ap()
    beta_in = nc.dram_tensor("beta_in", [128, 3, H], dt.bfloat16, kind="ExternalInput").ap()
    # stacked [demb(20); i-indicator(38)] and [w0d(20); alpha(38)]: the
    # delta and alpha terms land in one matmul
    dac_in = nc.dram_tensor("dac_in", [F + IPC, PPCB], dt.bfloat16, kind="ExternalInput").ap()
    w0da_in = nc.dram_tensor("w0da_in", [F + IPC, H], dt.bfloat16, kind="ExternalInput").ap()
    w0c_in = nc.dram_tensor("w0c_in", [128, KC8, H], dt.float8e4, kind="ExternalInput").ap()
    w1row = nc.dram_tensor("w1row", [1, H], dt.float32, kind="ExternalInput").ap()
    slow_out = nc.dram_tensor("slow", [128, PCH], dt.float32, kind="ExternalOutput").ap()

    with tile.TileContext(nc) as tc, ExitStack() as ctx:
        cst = ctx.enter_context(tc.tile_pool(name="cst", bufs=1))
        jidx_sb = cst.tile([128, PCH], dt.int32, tag="jidx_sb")
        nc.sync.dma_start(out=jidx_sb[:], in_=jidx.rearrange("(c p) one -> p (c one)", p=128))
        eit_sb = cst.tile([128, KC8, IPC], dt.bfloat16, tag="eit_sb")
        nc.scalar.dma_start(out=eit_sb[:], in_=eit_in[:, :, :])
        w0c_sb = cst.tile([128, KC8, H], dt.float8e4, tag="w0c_sb")
        nc.sync.dma_start(out=w0c_sb[:], in_=w0c_in[:, :, :])
        jind_sb = cst.tile([128, 3, PPCB], dt.bfloat16, tag="jind_sb")
        nc.scalar.dma_start(out=jind_sb[:], in_=jind_in[:, :, :])
        beta_sb = cst.tile([128, 3, H], dt.bfloat16, tag="beta_sb")
        nc.scalar.dma_start(out=beta_sb[:], in_=beta_in[:, :, :])
        dac_sb = cst.tile([F + IPC, PPCB], dt.bfloat16, tag="dac_sb")
        nc.scalar.dma_start(out=dac_sb[:], in_=dac_in[:, :])
        w0da_sb = cst.tile([F + IPC, H], dt.bfloat16, tag="w0da_sb")
        nc.scalar.dma_start(out=w0da_sb[:], in_=w0da_in[:, :])
        w1_bc = cst.tile([128, H], dt.float32, tag="w1_bc")
        nc.scalar.dma_start(out=w1_bc[:], in_=w1row[0:1, :].broadcast_to([128, H]))
        slow_sb = cst.tile([128, PCH], dt.float32, tag="slow_sb")

        ejT = cst.tile([128, KC8, PPCB], dt.bfloat16, tag="ejT")
        zT = cst.tile([128, KC8, PPCB], dt.float8e4, tag="zT")
        # pad pair columns (no eiT data there): zero once
        nc.vector.memset(zT[:, :, PPC:PPCB], 0.0)

        wk = ctx.enter_context(tc.tile_pool(name="wk", bufs=4))
        ps = ctx.enter_context(tc.tile_pool(name="ps", bufs=4, space="PSUM"))

        # pipeline: gather+transpose the pair blocks a 500-pair span needs,
        # z-mul that span (all d-chunks), repeat; matmul groups chase the
        # spans via tile deps.
        ZCH = 500  # 10 i-groups of 50
        zlims = list(range(0, PPC, ZCH)) + [PPC]
        done_pc = 0
        for zi in range(len(zlims) - 1):
            a, b = zlims[zi], zlims[zi + 1]
            need_pc = (b - 1) // 128
            for pc in range(done_pc, need_pc + 1):
                ej = wk.tile([128, DP8], dt.bfloat16, tag="ej", name="ej")
                nc.gpsimd.indirect_dma_start(
                    out=ej[:], out_offset=None, in_=embb[:, :],
                    in_offset=bass.IndirectOffsetOnAxis(ap=jidx_sb[:, pc:pc + 1], axis=0))
                eng = nc.sync if pc % 2 == 0 else nc.scalar
                eng.dma_start_transpose(ejT[:, :, pc * 128:(pc + 1) * 128], ej[:])
            done_pc = need_pc + 1
            ia, ib = a // K_CAP, b // K_CAP
            for kc in range(KC8):
                o3 = zT[:, kc, a:b].rearrange("p (i k) -> p i k", k=K_CAP)
                e3 = ejT[:, kc, a:b].rearrange("p (i k) -> p i k", k=K_CAP)
                w3 = eit_sb[:, kc, ia:ib][:, :, None].broadcast_to(
                    [128, ib - ia, K_CAP])
                nc.vector.tensor_mul(o3, e3, w3)

        # per pair-block: (delta+alpha) + beta + gamma in one PSUM group
        # (H split in 500-wide halves -- matmul output must fit one PSUM
        # bank), then relu * w1 + reduce via stt accum_out
        NH = H // 2
        half_acc = [cst.tile([128, PCH], dt.float32, tag=f"hacc{hh}", name=f"hacc{hh}")
                    for hh in range(2)]
        for pc in range(PCH):
            blk = slice(pc * 128, (pc + 1) * 128)
            for hh in range(2):
                hs = slice(hh * NH, (hh + 1) * NH)
                acc = ps.tile([128, NH], dt.float32, tag="acc", name="acc")
                nc.tensor.matmul(acc[:], lhsT=dac_sb[:, blk], rhs=w0da_sb[:, hs],
                                 start=True, stop=False, skip_group_check=True)
                for g in range(3):
                    nc.tensor.matmul(acc[:], lhsT=jind_sb[:, g, blk],
                                     rhs=beta_sb[:, g, hs],
                                     start=False, stop=False, skip_group_check=True)
                for g in range(KC8 // 2):
                    nc.tensor.matmul(acc[:], lhsT=zT[:, 2 * g:2 * g + 2, blk],
                                     rhs=w0c_sb[:, 2 * g:2 * g + 2, hs],
                                     perf_mode=PM.DoubleRow,
                                     start=False, stop=(g == KC8 // 2 - 1),
                                     skip_group_check=True)
                y = wk.tile([128, NH], dt.bfloat16, tag="y", name="y")
                nc.vector.scalar_tensor_tensor(
                    y[:], acc[:], 0.0, w1_bc[:, hs],
                    ALU.max, ALU.mult, accum_out=half_acc[hh][:, pc:pc + 1])
        nc.vector.tensor_add(slow_sb[:], half_acc[0][:], half_acc[1][:])
        nc.sync.dma_start(out=slow_out[:, :], in_=slow_sb[:])

    nc.compile()
    return nc


# ------------------------------------------------------------- host -----
_BUILT = {}
TRACE = False
PERF = {}


def _run(name, nc, in_maps, cores):
    try:
        res = run_bass_kernel_spmd(nc, in_maps, core_ids=cores, trace=TRACE)
    except ModuleNotFoundError:
        res = run_bass_kernel_spmd(nc, in_maps, core_ids=cores)
    if res.exec_time_ns is not None:
        PERF[name] = res.exec_time_ns
    return res


def _get(name, builder):
    if name not in _BUILT:
        _BUILT[name] = builder()
    return _BUILT[name]


def _bucket_dist(d):
    df = d.astype(np.float32)
    with np.errstate(divide="ignore"):
        log_idx = np.floor(np.log(np.maximum(df, np.float32(1.0))) /
                           np.float32(np.log(2.0))) + np.float32(3.0)
    comb = np.where(d <= 4, df, log_idx)
    return np.clip(comb, 0, 9).astype(np.int32)


def _nms(order, starts, ends, m):
    top = []
    ts = np.zeros(m, np.int64)
    te_ = np.zeros(m, np.int64)
    n = 0
    i = 0
    C = order.shape[0]
    while n < m and i < C:
        idx = order[i]
        s, e = starts[idx], ends[idx]
        if n:
            a = ts[:n]
            b = te_[:n]
            cross = (((s < a) & (e < b) & (e >= a)) | ((s > a) & (s <= b) & (e > b))).any()
        else:
            cross = False
        if not cross:
            top.append(idx)
            ts[n] = s
            te_[n] = e
            n += 1
        i += 1
    return np.asarray(top, np.int64)


def kernel(tokens_embed, Wh_w, Wh_b, width_emb, dist_emb,
           Sm_w0, Sm_b0, Sm_w1, Sm_b1,
           Smm_w0, Smm_b0, Smm_w1, Smm_b1,
           c2f_w, c2f_b, m, k):
    m = int(m)
    k = int(k)
    assert m == M_CAP and k == K_CAP, (m, k)
    f32 = np.float32
    bf16 = ml_dtypes.bfloat16
    f8 = ml_dtypes.float8_e4m3fn
    te = np.ascontiguousarray(tokens_embed, f32)
    cores = list(range(NCORES))

    # ---- host prep (index/pad/exp glue)
    te_pad = np.concatenate([te, np.repeat(te[-1:], TPAD - T, 0)], 0)
    tok = (te @ Wh_w.astype(f32))[:, 0] + f32(Wh_b[0])
    p_full = np.exp(tok.astype(f32)).astype(f32)
    p_pad = np.concatenate([p_full, np.zeros(TPAD - T, f32)])
    # den/r per (s, w): sequential fp32 prefix
    den = np.empty((T, W), f32)
    acc = p_full.copy()
    den[:, 0] = acc
    for w_ in range(1, W):
        nxt = np.concatenate([p_full[w_:], np.zeros(w_, f32)])
        acc = (acc + nxt).astype(f32)
        den[:, w_] = acc
    r_full = (f32(1.0) / den).astype(f32)
    # cumulative p*te for the head segment sums: cum[t] = sum_{t'<t} p*te
    cum_pte = np.zeros((T + 1, E), f32)
    np.cumsum((p_full[:, None] * te).astype(f32), axis=0, dtype=f32,
              out=cum_pte[1:])

    # L1 inputs
    w0abd_flat = np.concatenate([Sm_w0[0:E], Sm_w0[E:2 * E], Sm_w0[2 * E + F:]], 0).astype(f32)
    w0abd = np.ascontiguousarray(
        w0abd_flat.reshape(18, 128, 8, 125).transpose(2, 0, 1, 3))
    w0wd = np.concatenate([Sm_w0[2 * E:2 * E + F], Sm_b0[None, :]], 0).astype(f32)
    we_aug = np.concatenate([width_emb.T, np.ones((1, W))], 0).astype(f32)
    in1 = []
    for c in cores:
        t0 = c * TC
        in1.append(dict(
            te_loc=np.ascontiguousarray(te_pad[t0:t0 + COLS]),
            p_loc=np.ascontiguousarray(p_pad[None, t0:t0 + COLS]),
            r_loc=np.ascontiguousarray(r_full[t0:t0 + TC].reshape(1, SPC)),
            w0abd=w0abd, w0wd=w0wd, we_aug=we_aug,
            w1_in=Sm_w1.astype(f32)))
    nc1 = _get("l1", build_l1)
    res1 = _run("l1", nc1, in1, cores)
    scores = np.concatenate([res1.results[c]["score"][0] for c in cores])

    # ---- host: mask + NMS
    starts = np.repeat(np.arange(T, dtype=np.int64), W)
    widths = np.tile(np.arange(W, dtype=np.int64), T)
    ends = starts + widths
    valid = ends < T
    sc = np.where(valid, (scores + f32(Sm_b1[0])).astype(f32), NEG).astype(f32)
    order = np.argsort(-sc, kind="stable")
    top_idx = _nms(order, starts, ends, m)
    ts_, tw = starts[top_idx], widths[top_idx]
    tec = np.minimum(ts_ + tw, T - 1)
    top_score = sc[top_idx]

    # ---- host: span embedding assembly (indexing glue)
    head = ((cum_pte[tec + 1] - cum_pte[ts_]) *
            r_full[ts_, tw][:, None]).astype(f32)
    emb = np.zeros((M_CAP, DP), f32)
    emb[:, 0:E] = te[ts_]
    emb[:, E:2 * E] = te[tec]
    emb[:, 2 * E:3 * E] = head
    emb[:, 3 * E:3 * E + F] = width_emb.astype(f32)[tw]
    emb[:, _BIAS_C2F] = 1.0
    emb[:, _BIAS_B0] = 1.0
    embt_in = np.ascontiguousarray(emb.T)

    # ---- L2
    perm = _PERM
    c2f_pad = np.zeros((DP, DP), f32)
    c2f_pad[:D, :D] = c2f_w.astype(f32)[np.ix_(perm, perm)]
    c2f_pad[_BIAS_C2F, :D] = c2f_b.astype(f32)[perm]
    w0a_pad = np.zeros((DP, H), f32)
    w0a_pad[:D] = Smm_w0[0:D].astype(f32)[perm] * f32(S3)
    w0b_pad = np.zeros((DP, H), f32)
    w0b_pad[:D] = Smm_w0[D:2 * D].astype(f32)[perm] * f32(S3)
    w0b_pad[_BIAS_B0] = Smm_b0.astype(f32) * f32(S3)
    in2 = []
    for c in cores:
        hc0 = c * 125
        cs = [3 * c, 3 * c + 1, 3 * c + 2]
        c2fc = np.zeros((DP, 384), f32)
        eselc = np.zeros((128, 3, M_CAP), f32)
        for si, ch in enumerate(cs):
            if ch < KC:
                c2fc[:, si * 128:(si + 1) * 128] = c2f_pad[:, ch * 128:(ch + 1) * 128]
                eselc[:, si, :] = embt_in[ch * 128:(ch + 1) * 128, :]
        in2.append(dict(
            embt_in=embt_in, esel_in=eselc, c2f_c=c2fc,
            w0a_c=np.ascontiguousarray(w0a_pad[:, hc0:hc0 + 125]).astype(bf16),
            w0b_c=np.ascontiguousarray(w0b_pad[:, hc0:hc0 + 125]).astype(bf16)))
    nc2 = _get("l2", build_l2)
    res2 = _run("l2", nc2, in2, cores)
    pair = np.zeros((m, m), f32)
    for c in cores:
        pair += res2.results[c]["pair_part"]
    pair = pair.astype(f32)
    alpha16 = np.concatenate([res2.results[c]["alpha_part"] for c in cores], 1)
    beta16 = np.concatenate([res2.results[c]["beta_part"] for c in cores], 1)

    # ---- host: visibility + top-k
    offset = np.arange(m, dtype=np.int64)[:, None] - np.arange(m, dtype=np.int64)[None, :]
    vis = offset >= 1
    all_score = (np.where(vis, f32(0.0), NEG).astype(f32) + top_score[:, None]).astype(f32)
    all_score = (all_score + pair).astype(f32)
    ante_idx = np.argsort(-all_score, axis=1, kind="stable")[:, :k]
    fast = np.take_along_axis(all_score, ante_idx, axis=1).astype(f32)
    ante_off = np.take_along_axis(offset, ante_idx, axis=1)
    dbuck = _bucket_dist(ante_off)

    # ---- L3
    emb16 = np.zeros((MP, DP8), bf16)
    emb16[:m, :DP] = emb.astype(bf16)
    emb16_f = emb16.astype(f32)
    w0c_pad = np.zeros((DP8, H), f32)
    w0c_pad[:D] = Smm_w0[2 * D:3 * D].astype(f32)[perm] * f32(S3)
    w0c8 = np.ascontiguousarray(
        w0c_pad.reshape(KC8, 128, H).transpose(1, 0, 2)).astype(f8)
    w0d16 = (Smm_w0[3 * D:3 * D + F].astype(f32) * f32(S3)).astype(bf16)
    ind16 = np.zeros((IPC, PPCB), bf16)
    for r_ in range(IPC):
        ind16[r_, r_ * K_CAP:(r_ + 1) * K_CAP] = 1.0
    beta_pad = _pad_rows(np.asarray(beta16, bf16), 384)
    beta_c = np.ascontiguousarray(beta_pad.reshape(3, 128, H).transpose(1, 0, 2))
    alpha_pad = _pad_rows(np.asarray(alpha16, bf16), MP)
    in3 = []
    for c in cores:
        i0 = c * IPC
        jj = np.zeros(PPCB, np.int32)
        dmb = np.zeros((F, PPCB), f32)
        for r_ in range(IPC):
            gi = i0 + r_
            if gi < m:
                sl = slice(r_ * K_CAP, (r_ + 1) * K_CAP)
                jj[sl] = ante_idx[gi]
                dmb[:, sl] = dist_emb.astype(f32)[dbuck[gi]].T
        eit = np.ascontiguousarray(
            emb16_f[i0:i0 + IPC].reshape(IPC, KC8, 128).transpose(2, 1, 0)).astype(bf16)
        jind = np.zeros((128, 3, PPCB), bf16)
        jind[jj % 128, jj // 128, np.arange(PPCB)] = 1.0
        dac = np.concatenate([dmb.astype(bf16), ind16], 0)
        w0da = np.concatenate([w0d16, np.asarray(alpha_pad[i0:i0 + IPC])], 0)
        in3.append(dict(
            embb=emb16, jidx=jj[:, None], eit_in=eit,
            jind_in=jind, beta_in=beta_c,
            dac_in=np.ascontiguousarray(dac), w0da_in=np.ascontiguousarray(w0da),
            w0c_in=w0c8, w1row=np.ascontiguousarray(Smm_w1.astype(f32).T / f32(S3))))
    nc3 = _get("l3", build_l3)
    res3 = _run("l3", nc3, in3, cores)
    slow = np.zeros((m, k), f32)
    for c in cores:
        sl = np.ascontiguousarray(res3.results[c]["slow"].T).reshape(PPCB)
        i0 = c * IPC
        for r_ in range(IPC):
            gi = i0 + r_
            if gi < m:
                slow[gi] = sl[r_ * K_CAP:(r_ + 1) * K_CAP]
    slow = (slow + f32(Smm_b1[0])).astype(f32)
    return (fast + slow).astype(f32)


# revision 9
# speedup vs baseline: 1.3906x; 1.1055x over previous
"""CorefModel TRN2 kernel: 8-core SPMD Bass implementation.

Pipeline (3 device launches + host glue):
  L1: span scores  [C=24576] -- candidate-span axis sharded over 8 cores.
      Decomposed span FFNN: score = w1.relu(A[s] + B[s+w] + Wd[w] + b0
      + (sum_t p_t G[t]) / den), with A/B/G = te @ W0-slices computed once
      per token instead of per span (16x FLOP reduction).
  host: mask invalid, stable argsort, greedy non-crossing NMS -> top_idx;
      span embedding assembly (row gathers + head segment-sums from a
      p*te cumsum) -- pure indexing glue, O(m*D).
  L2: src = emb @ c2f + partial pair matrix (contraction-d sharded over
      cores, host sums; fp32 -- selection-critical), alpha/beta =
      emb @ W0a/b h-slices in bf16 (value path).
  host: visibility mask + top_score add, per-row stable top-k -> ante_idx,
      distance buckets.
  L3: pair FFNN slow score, pairs sharded over cores (38 i-rows x 50 each).
      gamma term via fp8 DoubleRow matmuls on zT = ejT * broadcast(eiT);
      ej gathered bf16 + DMA-transposed; alpha/beta/dist folded into the
      same PSUM accumulation via small bf16 selection matmuls; final
      relu+w1+reduce in one DVE scalar_tensor_tensor with accum_out.

All selection-critical arithmetic (L1 scores, L2 pair matrix) is fp32;
the slow-score value path uses bf16/fp8 (tolerates >1e-2 noise).
"""
import numpy as np
import ml_dtypes
from contextlib import ExitStack

import concourse.bass as bass
import concourse.mybir as mybir
import concourse.tile as tile
from concourse import bacc
from concourse.bass_utils import run_bass_kernel_spmd
from concourse.masks import make_identity

dt = mybir.dt
AF = mybir.ActivationFunctionType
ALU = mybir.AluOpType
PM = mybir.MatmulPerfMode

NCORES = 8
T, E, W, F, H = 1536, 768, 16, 20, 1000
D = 2 * E + F + E              # 2324
DP = 2432                      # D padded to 19*128 (order: start,end,head,width,pad)
KC = DP // 128                 # 19 d-chunks
DP8 = 2560                     # L3 pair-term d padded to 20*128 (for fp8 DoubleRow)
KC8 = DP8 // 128               # 20
TC = T // NCORES               # 192 starts per core
COLS = 256                     # per-core t-window (t0 .. t0+256), te padded to 1600
TPAD = T + 64                  # 1600
SPC = TC * W                   # 3072 spans per core
M_CAP, K_CAP = 300, 50
MP = 304                       # m padded to 8*38
IPC = MP // NCORES             # 38 i-rows per core
PPC = IPC * K_CAP              # 1900 pairs per core
PPCB = 1920                    # padded to 15*128 tile blocks
PCH = PPCB // 128              # 15 pair chunks
S3 = 64.0                      # L3 value-path scale (fp8 range), undone via w1
NEG = np.float32(-1e30)

# se' feature permutation: old order [start(768) end(768) width(20) head(768)],
# new order [start end head width pad(108)]
_PERM = np.concatenate([
    np.arange(0, 2 * E),                   # start, end
    np.arange(2 * E + F, 2 * E + F + E),   # head
    np.arange(2 * E, 2 * E + F),           # width
]).astype(np.int64)            # maps new[0:2324] -> old index

_BIAS_C2F = 2430               # emb slot fixed to 1.0 (c2f bias row)
_BIAS_B0 = 2431                # emb slot fixed to 1.0 (Smm_b0 row via w0b)


def _pad_rows(a, rows):
    out = np.zeros((rows,) + a.shape[1:], a.dtype)
    out[:a.shape[0]] = a
    return out


# ---------------------------------------------------------------- L1 ----
def build_l1():
    nc = bacc.Bacc(trn_type="TRN2", target_bir_lowering=False, debug=False)
    te_loc = nc.dram_tensor("te_loc", [COLS, E], dt.float32, kind="ExternalInput").ap()
    p_loc = nc.dram_tensor("p_loc", [1, COLS], dt.float32, kind="ExternalInput").ap()
    r_loc = nc.dram_tensor("r_loc", [1, SPC], dt.float32, kind="ExternalInput").ap()
    w0abd = nc.dram_tensor("w0abd", [8, 18, 128, H // 8], dt.float32, kind="ExternalInput").ap()
    w0wd = nc.dram_tensor("w0wd", [F + 1, H], dt.float32, kind="ExternalInput").ap()
    we_aug = nc.dram_tensor("we_aug", [F + 1, W], dt.float32, kind="ExternalInput").ap()
    w1_in = nc.dram_tensor("w1_in", [H, 1], dt.float32, kind="ExternalInput").ap()
    score_out = nc.dram_tensor("score", [1, SPC], dt.float32, kind="ExternalOutput").ap()

    HC = 8
    HB = H // HC  # 125
    NT = 512

    with tile.TileContext(nc) as tc, ExitStack() as ctx:
        cst = ctx.enter_context(tc.tile_pool(name="cst", bufs=1))
        ps = ctx.enter_context(tc.tile_pool(name="ps", bufs=2, space="PSUM"))
        scps = ctx.enter_context(tc.tile_pool(name="scps", bufs=1, space="PSUM"))
        npool = ctx.enter_context(tc.tile_pool(name="npool", bufs=3))
        abp = ctx.enter_context(tc.tile_pool(name="abp", bufs=3))
        comb = ctx.enter_context(tc.tile_pool(name="comb", bufs=3))

        # --- constant loads (G-part of W first so PE starts early)
        te_sb = cst.tile([128, 2, E], dt.float32, tag="te_sb")
        nc.sync.dma_start(out=te_sb[:], in_=te_loc.rearrange("(c p) e -> p c e", p=128))
        ident = cst.tile([128, 128], dt.float32, tag="ident")
        make_identity(nc, ident)
        wdsb = cst.tile([F + 1, H], dt.float32, tag="wdsb")
        nc.sync.dma_start(out=wdsb[:], in_=w0wd[:, :])
        wesb = cst.tile([F + 1, W], dt.float32, tag="wesb")
        nc.sync.dma_start(out=wesb[:], in_=we_aug[:, :])
        w1sb = cst.tile([HB, HC], dt.float32, tag="w1sb")
        nc.sync.dma_start(out=w1sb[:], in_=w1_in.rearrange("(c p) one -> p (c one)", p=HB))
        p_bc = cst.tile([128, COLS], dt.float32, tag="p_bc")
        nc.sync.dma_start(out=p_bc[:], in_=p_loc[0:1, :].broadcast_to([128, COLS]))
        r_bc = cst.tile([128, SPC], dt.float32, tag="r_bc")
        nc.sync.dma_start(out=r_bc[:], in_=r_loc[0:1, :].broadcast_to([128, SPC]))

        # --- te^T tiles [128e, 256t] x6
        teT = []
        for ec in range(6):
            t_ = cst.tile([128, COLS], dt.float32, tag=f"teT{ec}", name=f"teT{ec}")
            teT.append(t_)
        for tcki in range(2):
            for ec in range(6):
                pt = ps.tile([128, 128], dt.float32, tag="acc")
                nc.tensor.transpose(out=pt[:], in_=te_sb[:, tcki, ec * 128:(ec + 1) * 128],
                                    identity=ident[:])
                nc.scalar.copy(teT[ec][:, tcki * 128:(tcki + 1) * 128], pt[:])

        sc_ps = [scps.tile([1, NT], dt.float32, tag=f"sc{j}", name=f"sc{j}")
                 for j in range(SPC // NT)]

        wpool = ctx.enter_context(tc.tile_pool(name="wpool", bufs=3))

        def mat_tile(whc, m3, tag):
            t_ = abp.tile([HB, COLS], dt.float32, tag=tag, name=tag)
            acc = ps.tile([HB, COLS], dt.float32, tag="acc", name="acc")
            for ec in range(6):
                nc.tensor.matmul(acc[:],
                                 lhsT=whc[:, m3 * 6 + ec, :],
                                 rhs=teT[ec][:],
                                 start=(ec == 0), stop=(ec == 5))
            nc.scalar.copy(t_[:], acc[:])
            return t_

        # --- per-h-chunk pipeline (W streamed per h-chunk, G chunks first)
        for hc in range(HC):
            whc = wpool.tile([128, 18, HB], dt.float32, tag="whc", name="whc")
            nc.sync.dma_start(out=whc[:], in_=w0abd[hc].rearrange("c p h -> p c h"))
            g_t = mat_tile(whc, 2, "g_t")
            pg = npool.tile([HB, COLS], dt.float32, tag="pg", name="pg")
            nc.vector.tensor_mul(pg[:], g_t[:], p_bc[:HB, :])
            numw = npool.tile([HB, SPC], dt.float32, tag="numw", name="numw")
            n3 = numw[:].rearrange("p (s w) -> p s w", w=W)
            nc.gpsimd.tensor_copy(n3[:, :, 0], pg[:, 0:TC])
            for w_ in range(1, W):
                nc.gpsimd.tensor_add(n3[:, :, w_], n3[:, :, w_ - 1], pg[:, w_:w_ + TC])
            a_t = mat_tile(whc, 0, "a_t")
            b_t = mat_tile(whc, 1, "b_t")
            wd_ps = ps.tile([HB, W], dt.float32, tag="acc", name="wdacc")
            nc.tensor.matmul(wd_ps[:], lhsT=wdsb[:, hc * HB:(hc + 1) * HB],
                             rhs=wesb[:], start=True, stop=True)
            wd_t = abp.tile([HB, W], dt.float32, tag="wd_t", name="wd_t")
            nc.scalar.copy(wd_t[:], wd_ps[:])

            # full-width combine: x = numW*r + A + B  (three wide DVE ops)
            x = comb.tile([HB, SPC], dt.float32, tag="x", name="x")
            nc.vector.tensor_mul(x[:], numw[:], r_bc[:HB, :])
            x3 = x[:].rearrange("p (s w) -> p s w", w=W)
            a_op = a_t[:, 0:TC][:, :, None].broadcast_to([HB, TC, W])
            nc.vector.tensor_add(x3, x3, a_op)
            bt = b_t[:]
            b_op = bass.AP(bt.tensor, bt.offset, [bt.ap[0], [1, TC], [1, W]])
            nc.vector.tensor_add(x3, x3, b_op)
            # wd folded into relu bias, one ACT op per width
            y = comb.tile([HB, SPC], dt.float32, tag="y", name="y")
            y3 = y[:].rearrange("p (s w) -> p s w", w=W)
            for w_ in range(W):
                nc.scalar.activation(y3[:, :, w_], x3[:, :, w_], AF.Relu,
                                     bias=wd_t[:, w_:w_ + 1])
            for j in range(SPC // NT):
                nc.tensor.matmul(sc_ps[j][:], lhsT=w1sb[:, hc:hc + 1],
                                 rhs=y[:, j * NT:(j + 1) * NT],
                                 start=(hc == 0), stop=(hc == HC - 1))

        for j in range(SPC // NT):
            so = comb.tile([1, NT], dt.float32, tag="so", name="so")
            nc.vector.tensor_copy(so[:], sc_ps[j][:])
            nc.sync.dma_start(out=score_out[:, j * NT:(j + 1) * NT], in_=so[:])

    nc.compile()
    return nc


# ---------------------------------------------------------------- L2 ----
def build_l2():
    nc = bacc.Bacc(trn_type="TRN2", target_bir_lowering=False, debug=False)
    embt_in = nc.dram_tensor("embt_in", [DP, M_CAP], dt.float32, kind="ExternalInput").ap()
    esel_in = nc.dram_tensor("esel_in", [128, 3, M_CAP], dt.float32, kind="ExternalInput").ap()
    c2f_c = nc.dram_tensor("c2f_c", [DP, 384], dt.float32, kind="ExternalInput").ap()
    w0a_c = nc.dram_tensor("w0a_c", [DP, 125], dt.bfloat16, kind="ExternalInput").ap()
    w0b_c = nc.dram_tensor("w0b_c", [DP, 125], dt.bfloat16, kind="ExternalInput").ap()

    pair_out = nc.dram_tensor("pair_part", [M_CAP, M_CAP], dt.float32, kind="ExternalOutput").ap()
    a_out = nc.dram_tensor("alpha_part", [M_CAP, 125], dt.bfloat16, kind="ExternalOutput").ap()
    b_out = nc.dram_tensor("beta_part", [M_CAP, 125], dt.bfloat16, kind="ExternalOutput").ap()

    MM = M_CAP
    with tile.TileContext(nc) as tc, ExitStack() as ctx:
        cst = ctx.enter_context(tc.tile_pool(name="cst", bufs=1))
        ps = ctx.enter_context(tc.tile_pool(name="ps", bufs=2, space="PSUM"))

        ident = cst.tile([128, 128], dt.float32, tag="ident")
        make_identity(nc, ident)
        embT = cst.tile([128, KC, MM], dt.float32, tag="embT")
        embT16 = cst.tile([128, KC, MM], dt.bfloat16, tag="embT16")
        c2f_sb = cst.tile([128, KC, 384], dt.float32, tag="c2f_sb")
        embt3 = embt_in.rearrange("(c p) s -> p c s", p=128)
        c2f3 = c2f_c.rearrange("(c p) s -> p c s", p=128)
        # chunked, queue-split loads so the src matmul chain starts on the
        # first chunk instead of a whole-tile barrier
        for ck in range(KC):
            nc.scalar.dma_start(out=embT[:, ck, :], in_=embt3[:, ck, :])
            nc.sync.dma_start(out=c2f_sb[:, ck, :], in_=c2f3[:, ck, :])
            nc.vector.tensor_copy(embT16[:, ck, :], embT[:, ck, :])
        esel = cst.tile([128, 3, MM], dt.float32, tag="esel")
        nc.scalar.dma_start(out=esel[:], in_=esel_in[:, :, :])
        w0a_sb = cst.tile([128, KC, 125], dt.bfloat16, tag="w0a_sb")
        nc.sync.dma_start(out=w0a_sb[:], in_=w0a_c.rearrange("(c p) s -> p c s", p=128))
        w0b_sb = cst.tile([128, KC, 125], dt.bfloat16, tag="w0b_sb")
        nc.sync.dma_start(out=w0b_sb[:], in_=w0b_c.rearrange("(c p) s -> p c s", p=128))

        # --- src slots + partial pair
        slotp = ctx.enter_context(tc.tile_pool(name="slotp", bufs=1))
        srcT = []
        for si in range(3):
            t_ = slotp.tile([128, MM], dt.float32, tag=f"srcT{si}", name=f"srcT{si}")
            srcT.append(t_)
        for si in range(3):
            acc = ps.tile([128, MM], dt.float32, tag="acc_ps")
            for kc in range(KC):
                nc.tensor.matmul(acc[:], lhsT=c2f_sb[:, kc, si * 128:(si + 1) * 128],
                                 rhs=embT[:, kc, :], start=(kc == 0), stop=(kc == KC - 1))
            nc.scalar.copy(srcT[si][:], acc[:])
        prow = ctx.enter_context(tc.tile_pool(name="prow", bufs=2))
        for it in range(3):
            inn = min(128, MM - it * 128)
            acc = ps.tile([128, MM], dt.float32, tag="acc_ps")
            for si in range(3):
                nc.tensor.matmul(acc[:inn, :], lhsT=srcT[si][:, it * 128:it * 128 + inn],
                                 rhs=esel[:, si, :], start=(si == 0), stop=(si == 2))
            pr = prow.tile([128, MM], dt.float32, tag="pr")
            nc.scalar.copy(pr[:inn, :], acc[:inn, :])
            nc.sync.dma_start(out=pair_out[it * 128:it * 128 + inn, :], in_=pr[:inn, :])

        # --- alpha/beta h-chunk (125 cols of H per core), bf16 value path
        abrow = ctx.enter_context(tc.tile_pool(name="abrow", bufs=2))
        for wsb_, out_ in ((w0a_sb, a_out), (w0b_sb, b_out)):
            acc = ps.tile([125, MM], dt.float32, tag="acc_ps")
            for kc in range(KC):
                nc.tensor.matmul(acc[:], lhsT=wsb_[:, kc, :], rhs=embT16[:, kc, :],
                                 start=(kc == 0), stop=(kc == KC - 1))
            abT = abrow.tile([125, MM], dt.float32, tag="abT")
            nc.scalar.copy(abT[:], acc[:])
            for g in range(3):
                gn = min(128, MM - g * 128)
                pt = ps.tile([128, 125], dt.float32, tag="tr_ps")
                nc.tensor.transpose(out=pt[:gn, :], in_=abT[:, g * 128:g * 128 + gn],
                                    identity=ident[:125, :125])
                ab_r = abrow.tile([128, 125], dt.bfloat16, tag="ab_r")
                nc.scalar.copy(ab_r[:gn, :], pt[:gn, :])
                nc.sync.dma_start(out=out_[g * 128:g * 128 + gn, :], in_=ab_r[:gn, :])

    nc.compile()
    return nc


# ---------------------------------------------------------------- L3 ----
def build_l3():
    nc = bacc.Bacc(trn_type="TRN2", target_bir_lowering=False, debug=False)
    embb = nc.dram_tensor("embb", [MP, DP8], dt.bfloat16, kind="ExternalInput").ap()
    jidx = nc.dram_tensor("jidx", [PPCB, 1], dt.int32, kind="ExternalInput").ap()
    eit_in = nc.dram_tensor("eit_in", [128, KC8, IPC], dt.bfloat16, kind="ExternalInput").ap()
    jind_in = nc.dram_tensor("jind_in", [128, 3, PPCB], dt.bfloat16, kind="ExternalInput").ap()
    beta_in = nc.dram_tensor("beta_in", [128, 3, H], dt.bfloat16, kind="ExternalInput").ap()
    # stacked [demb(20); i-indicator(38)] and [w0d(20); alpha(38)]: the
    # delta and alpha terms land in one matmul
    dac_in = nc.dram_tensor("dac_in", [F + IPC, PPCB], dt.bfloat16, kind="ExternalInput").ap()
    w0da_in = nc.dram_tensor("w0da_in", [F + IPC, H], dt.bfloat16, kind="ExternalInput").ap()
    w0c_in = nc.dram_tensor("w0c_in", [128, KC8, H], dt.float8e4, kind="ExternalInput").ap()
    w1row = nc.dram_tensor("w1row", [1, H], dt.float32, kind="ExternalInput").ap()
    slow_out = nc.dram_tensor("slow", [128, PCH], dt.float32, kind="ExternalOutput").ap()

    with tile.TileContext(nc) as tc, ExitStack() as ctx:
        cst = ctx.enter_context(tc.tile_pool(name="cst", bufs=1))
        jidx_sb = cst.tile([128, PCH], dt.int32, tag="jidx_sb")
        nc.sync.dma_start(out=jidx_sb[:], in_=jidx.rearrange("(c p) one -> p (c one)", p=128))
        eit_sb = cst.tile([128, KC8, IPC], dt.bfloat16, tag="eit_sb")
        nc.scalar.dma_start(out=eit_sb[:], in_=eit_in[:, :, :])
        w0c_sb = cst.tile([128, KC8, H], dt.float8e4, tag="w0c_sb")
        nc.sync.dma_start(out=w0c_sb[:], in_=w0c_in[:, :, :])
        jind_sb = cst.tile([128, 3, PPCB], dt.bfloat16, tag="jind_sb")
        nc.scalar.dma_start(out=jind_sb[:], in_=jind_in[:, :, :])
        beta_sb = cst.tile([128, 3, H], dt.bfloat16, tag="beta_sb")
        nc.scalar.dma_start(out=beta_sb[:], in_=beta_in[:, :, :])
        dac_sb = cst.tile([F + IPC, PPCB], dt.bfloat16, tag="dac_sb")
        nc.scalar.dma_start(out=dac_sb[:], in_=dac_in[:, :])
        w0da_sb = cst.tile([F + IPC, H], dt.bfloat16, tag="w0da_sb")
        nc.scalar.dma_start(out=w0da_sb[:], in_=w0da_in[:, :])
        w1_bc = cst.tile([128, H], dt.float32, tag="w1_bc")
        nc.scalar.dma_start(out=w1_bc[:], in_=w1row[0:1, :].broadcast_to([128, H]))
        slow_sb = cst.tile([128, PCH], dt.float32, tag="slow_sb")

        ejT = cst.tile([128, KC8, PPCB], dt.bfloat16, tag="ejT")
        zT = cst.tile([128, KC8, PPCB], dt.float8e4, tag="zT")
        # pad pair columns (no eiT data there): zero once
        nc.vector.memset(zT[:, :, PPC:PPCB], 0.0)

        wk = ctx.enter_context(tc.tile_pool(name="wk", bufs=4))
        ps = ctx.enter_context(tc.tile_pool(name="ps", bufs=8, space="PSUM"))

        NH = H // 2
        half_acc = [cst.tile([128, PCH], dt.float32, tag=f"hacc{hh}", name=f"hacc{hh}")
                    for hh in range(2)]

        def mm_group(pc, hh):
            # (delta+alpha) + beta + gamma in one PSUM group (H in 500-wide
            # halves: matmul output must fit one PSUM bank), then
            # relu * w1 + reduce via stt accum_out
            blk = slice(pc * 128, (pc + 1) * 128)
            hs = slice(hh * NH, (hh + 1) * NH)
            acc = ps.tile([128, NH], dt.float32, tag="acc", name="acc")
            nc.tensor.matmul(acc[:], lhsT=dac_sb[:, blk], rhs=w0da_sb[:, hs],
                             start=True, stop=False, skip_group_check=True)
            for g in range(3):
                nc.tensor.matmul(acc[:], lhsT=jind_sb[:, g, blk],
                                 rhs=beta_sb[:, g, hs],
                                 start=False, stop=False, skip_group_check=True)
            for g in range(KC8 // 2):
                nc.tensor.matmul(acc[:], lhsT=zT[:, 2 * g:2 * g + 2, blk],
                                 rhs=w0c_sb[:, 2 * g:2 * g + 2, hs],
                                 perf_mode=PM.DoubleRow,
                                 start=False, stop=(g == KC8 // 2 - 1),
                                 skip_group_check=True)
            y = wk.tile([128, NH], dt.bfloat16, tag="y", name="y")
            nc.vector.scalar_tensor_tensor(
                y[:], acc[:], 0.0, w1_bc[:, hs],
                ALU.max, ALU.mult, accum_out=half_acc[hh][:, pc:pc + 1])

        # pipeline: gather+transpose the pair blocks a 500-pair span needs,
        # z-mul that span (all d-chunks), then issue the matmul groups the
        # previous span enabled (keeps stt's interleaved in the DVE queue so
        # PSUM banks recycle while later spans are still multiplying).
        ZCH = 500  # 10 i-groups of 50
        zlims = list(range(0, PPC, ZCH)) + [PPC]
        done_pc = 0
        grp_done = 0
        for zi in range(len(zlims) - 1):
            a, b = zlims[zi], zlims[zi + 1]
            need_pc = (b - 1) // 128
            for pc in range(done_pc, need_pc + 1):
                ej = wk.tile([128, DP8], dt.bfloat16, tag="ej", name="ej")
                nc.gpsimd.indirect_dma_start(
                    out=ej[:], out_offset=None, in_=embb[:, :],
                    in_offset=bass.IndirectOffsetOnAxis(ap=jidx_sb[:, pc:pc + 1], axis=0))
                eng = nc.sync if pc % 2 == 0 else nc.scalar
                eng.dma_start_transpose(ejT[:, :, pc * 128:(pc + 1) * 128], ej[:])
            done_pc = need_pc + 1
            ia, ib = a // K_CAP, b // K_CAP
            for kc in range(KC8):
                o3 = zT[:, kc, a:b].rearrange("p (i k) -> p i k", k=K_CAP)
                e3 = ejT[:, kc, a:b].rearrange("p (i k) -> p i k", k=K_CAP)
                w3 = eit_sb[:, kc, ia:ib][:, :, None].broadcast_to(
                    [128, ib - ia, K_CAP])
                nc.vector.tensor_mul(o3, e3, w3)
            # pair blocks fully covered by spans processed so far
            ready_pc = b // 128
            for pc in range(grp_done, ready_pc):
                mm_group(pc, 0)
                mm_group(pc, 1)
            grp_done = ready_pc
        for pc in range(grp_done, PCH):
            mm_group(pc, 0)
            mm_group(pc, 1)
        nc.vector.tensor_add(slow_sb[:], half_acc[0][:], half_acc[1][:])
        nc.sync.dma_start(out=slow_out[:, :], in_=slow_sb[:])

    nc.compile()
    return nc


# ------------------------------------------------------------- host -----
_BUILT = {}
TRACE = False
PERF = {}


def _run(name, nc, in_maps, cores):
    try:
        res = run_bass_kernel_spmd(nc, in_maps, core_ids=cores, trace=TRACE)
    except ModuleNotFoundError:
        res = run_bass_kernel_spmd(nc, in_maps, core_ids=cores)
    if res.exec_time_ns is not None:
        PERF[name] = res.exec_time_ns
    return res


def _get(name, builder):
    if name not in _BUILT:
        _BUILT[name] = builder()
    return _BUILT[name]


def _bucket_dist(d):
    df = d.astype(np.float32)
    with np.errstate(divide="ignore"):
        log_idx = np.floor(np.log(np.maximum(df, np.float32(1.0))) /
                           np.float32(np.log(2.0))) + np.float32(3.0)
    comb = np.where(d <= 4, df, log_idx)
    return np.clip(comb, 0, 9).astype(np.int32)


def _nms(order, starts, ends, m):
    top = []
    ts = np.zeros(m, np.int64)
    te_ = np.zeros(m, np.int64)
    n = 0
    i = 0
    C = order.shape[0]
    while n < m and i < C:
        idx = order[i]
        s, e = starts[idx], ends[idx]
        if n:
            a = ts[:n]
            b = te_[:n]
            cross = (((s < a) & (e < b) & (e >= a)) | ((s > a) & (s <= b) & (e > b))).any()
        else:
            cross = False
        if not cross:
            top.append(idx)
            ts[n] = s
            te_[n] = e
            n += 1
        i += 1
    return np.asarray(top, np.int64)


def kernel(tokens_embed, Wh_w, Wh_b, width_emb, dist_emb,
           Sm_w0, Sm_b0, Sm_w1, Sm_b1,
           Smm_w0, Smm_b0, Smm_w1, Smm_b1,
           c2f_w, c2f_b, m, k):
    m = int(m)
    k = int(k)
    assert m == M_CAP and k == K_CAP, (m, k)
    f32 = np.float32
    bf16 = ml_dtypes.bfloat16
    f8 = ml_dtypes.float8_e4m3fn
    te = np.ascontiguousarray(tokens_embed, f32)
    cores = list(range(NCORES))

    # ---- host prep (index/pad/exp glue)
    te_pad = np.concatenate([te, np.repeat(te[-1:], TPAD - T, 0)], 0)
    tok = (te @ Wh_w.astype(f32))[:, 0] + f32(Wh_b[0])
    p_full = np.exp(tok.astype(f32)).astype(f32)
    p_pad = np.concatenate([p_full, np.zeros(TPAD - T, f32)])
    # den/r per (s, w): sequential fp32 prefix
    den = np.empty((T, W), f32)
    acc = p_full.copy()
    den[:, 0] = acc
    for w_ in range(1, W):
        nxt = np.concatenate([p_full[w_:], np.zeros(w_, f32)])
        acc = (acc + nxt).astype(f32)
        den[:, w_] = acc
    r_full = (f32(1.0) / den).astype(f32)
    # cumulative p*te for the head segment sums: cum[t] = sum_{t'<t} p*te
    cum_pte = np.zeros((T + 1, E), f32)
    np.cumsum((p_full[:, None] * te).astype(f32), axis=0, dtype=f32,
              out=cum_pte[1:])

    # L1 inputs
    w0abd_flat = np.concatenate([Sm_w0[0:E], Sm_w0[E:2 * E], Sm_w0[2 * E + F:]], 0).astype(f32)
    w0abd = np.ascontiguousarray(
        w0abd_flat.reshape(18, 128, 8, 125).transpose(2, 0, 1, 3))
    w0wd = np.concatenate([Sm_w0[2 * E:2 * E + F], Sm_b0[None, :]], 0).astype(f32)
    we_aug = np.concatenate([width_emb.T, np.ones((1, W))], 0).astype(f32)
    in1 = []
    for c in cores:
        t0 = c * TC
        in1.append(dict(
            te_loc=np.ascontiguousarray(te_pad[t0:t0 + COLS]),
            p_loc=np.ascontiguousarray(p_pad[None, t0:t0 + COLS]),
            r_loc=np.ascontiguousarray(r_full[t0:t0 + TC].reshape(1, SPC)),
            w0abd=w0abd, w0wd=w0wd, we_aug=we_aug,
            w1_in=Sm_w1.astype(f32)))
    nc1 = _get("l1", build_l1)
    res1 = _run("l1", nc1, in1, cores)
    scores = np.concatenate([res1.results[c]["score"][0] for c in cores])

    # ---- host: mask + NMS
    starts = np.repeat(np.arange(T, dtype=np.int64), W)
    widths = np.tile(np.arange(W, dtype=np.int64), T)
    ends = starts + widths
    valid = ends < T
    sc = np.where(valid, (scores + f32(Sm_b1[0])).astype(f32), NEG).astype(f32)
    order = np.argsort(-sc, kind="stable")
    top_idx = _nms(order, starts, ends, m)
    ts_, tw = starts[top_idx], widths[top_idx]
    tec = np.minimum(ts_ + tw, T - 1)
    top_score = sc[top_idx]

    # ---- host: span embedding assembly (indexing glue)
    head = ((cum_pte[tec + 1] - cum_pte[ts_]) *
            r_full[ts_, tw][:, None]).astype(f32)
    emb = np.zeros((M_CAP, DP), f32)
    emb[:, 0:E] = te[ts_]
    emb[:, E:2 * E] = te[tec]
    emb[:, 2 * E:3 * E] = head
    emb[:, 3 * E:3 * E + F] = width_emb.astype(f32)[tw]
    emb[:, _BIAS_C2F] = 1.0
    emb[:, _BIAS_B0] = 1.0
    embt_in = np.ascontiguousarray(emb.T)

    # ---- L2
    perm = _PERM
    c2f_pad = np.zeros((DP, DP), f32)
    c2f_pad[:D, :D] = c2f_w.astype(f32)[np.ix_(perm, perm)]
    c2f_pad[_BIAS_C2F, :D] = c2f_b.astype(f32)[perm]
    w0a_pad = np.zeros((DP, H), f32)
    w0a_pad[:D] = Smm_w0[0:D].astype(f32)[perm] * f32(S3)
    w0b_pad = np.zeros((DP, H), f32)
    w0b_pad[:D] = Smm_w0[D:2 * D].astype(f32)[perm] * f32(S3)
    w0b_pad[_BIAS_B0] = Smm_b0.astype(f32) * f32(S3)
    in2 = []
    for c in cores:
        hc0 = c * 125
        cs = [3 * c, 3 * c + 1, 3 * c + 2]
        c2fc = np.zeros((DP, 384), f32)
        eselc = np.zeros((128, 3, M_CAP), f32)
        for si, ch in enumerate(cs):
            if ch < KC:
                c2fc[:, si * 128:(si + 1) * 128] = c2f_pad[:, ch * 128:(ch + 1) * 128]
                eselc[:, si, :] = embt_in[ch * 128:(ch + 1) * 128, :]
        in2.append(dict(
            embt_in=embt_in, esel_in=eselc, c2f_c=c2fc,
            w0a_c=np.ascontiguousarray(w0a_pad[:, hc0:hc0 + 125]).astype(bf16),
            w0b_c=np.ascontiguousarray(w0b_pad[:, hc0:hc0 + 125]).astype(bf16)))
    nc2 = _get("l2", build_l2)
    res2 = _run("l2", nc2, in2, cores)
    pair = np.zeros((m, m), f32)
    for c in cores:
        pair += res2.results[c]["pair_part"]
    pair = pair.astype(f32)
    alpha16 = np.concatenate([res2.results[c]["alpha_part"] for c in cores], 1)
    beta16 = np.concatenate([res2.results[c]["beta_part"] for c in cores], 1)

    # ---- host: visibility + top-k
    offset = np.arange(m, dtype=np.int64)[:, None] - np.arange(m, dtype=np.int64)[None, :]
    vis = offset >= 1
    all_score = (np.where(vis, f32(0.0), NEG).astype(f32) + top_score[:, None]).astype(f32)
    all_score = (all_score + pair).astype(f32)
    ante_idx = np.argsort(-all_score, axis=1, kind="stable")[:, :k]
    fast = np.take_along_axis(all_score, ante_idx, axis=1).astype(f32)
    ante_off = np.take_along_axis(offset, ante_idx, axis=1)
    dbuck = _bucket_dist(ante_off)

    # ---- L3
    emb16 = np.zeros((MP, DP8), bf16)
    emb16[:m, :DP] = emb.astype(bf16)
    emb16_f = emb16.astype(f32)
    w0c_pad = np.zeros((DP8, H), f32)
    w0c_pad[:D] = Smm_w0[2 * D:3 * D].astype(f32)[perm] * f32(S3)
    w0c8 = np.ascontiguousarray(
        w0c_pad.reshape(KC8, 128, H).transpose(1, 0, 2)).astype(f8)
    w0d16 = (Smm_w0[3 * D:3 * D + F].astype(f32) * f32(S3)).astype(bf16)
    ind16 = np.zeros((IPC, PPCB), bf16)
    for r_ in range(IPC):
        ind16[r_, r_ * K_CAP:(r_ + 1) * K_CAP] = 1.0
    beta_pad = _pad_rows(np.asarray(beta16, bf16), 384)
    beta_c = np.ascontiguousarray(beta_pad.reshape(3, 128, H).transpose(1, 0, 2))
    alpha_pad = _pad_rows(np.asarray(alpha16, bf16), MP)
    in3 = []
    for c in cores:
        i0 = c * IPC
        jj = np.zeros(PPCB, np.int32)
        dmb = np.zeros((F, PPCB), f32)
        for r_ in range(IPC):
            gi = i0 + r_
            if gi < m:
                sl = slice(r_ * K_CAP, (r_ + 1) * K_CAP)
                jj[sl] = ante_idx[gi]
                dmb[:, sl] = dist_emb.astype(f32)[dbuck[gi]].T
        eit = np.ascontiguousarray(
            emb16_f[i0:i0 + IPC].reshape(IPC, KC8, 128).transpose(2, 1, 0)).astype(bf16)
        jind = np.zeros((128, 3, PPCB), bf16)
        jind[jj % 128, jj // 128, np.arange(PPCB)] = 1.0
        dac = np.concatenate([dmb.astype(bf16), ind16], 0)
        w0da = np.concatenate([w0d16, np.asarray(alpha_pad[i0:i0 + IPC])], 0)
        in3.append(dict(
            embb=emb16, jidx=jj[:, None], eit_in=eit,
            jind_in=jind, beta_in=beta_c,
            dac_in=np.ascontiguousarray(dac), w0da_in=np.ascontiguousarray(w0da),
            w0c_in=w0c8, w1row=np.ascontiguousarray(Smm_w1.astype(f32).T / f32(S3))))
    nc3 = _get("l3", build_l3)
    res3 = _run("l3", nc3, in3, cores)
    slow = np.zeros((m, k), f32)
    for c in cores:
        sl = np.ascontiguousarray(res3.results[c]["slow"].T).reshape(PPCB)
        i0 = c * IPC
        for r_ in range(IPC):
            gi = i0 + r_
            if gi < m:
                slow[gi] = sl[r_ * K_CAP:(r_ + 1) * K_CAP]
    slow = (slow + f32(Smm_b1[0])).astype(f32)
    return (fast + slow).astype(f32)


# revision 11
# speedup vs baseline: 1.3971x; 1.0046x over previous
"""CorefModel TRN2 kernel: 8-core SPMD Bass implementation.

Pipeline (3 device launches + host glue):
  L1: span scores  [C=24576] -- candidate-span axis sharded over 8 cores.
      Decomposed span FFNN: score = w1.relu(A[s] + B[s+w] + Wd[w] + b0
      + (sum_t p_t G[t]) / den), with A/B/G = te @ W0-slices computed once
      per token instead of per span (16x FLOP reduction).
  host: mask invalid, stable argsort, greedy non-crossing NMS -> top_idx;
      span embedding assembly (row gathers + head segment-sums from a
      p*te cumsum) -- pure indexing glue, O(m*D).
  L2: src = emb @ c2f + partial pair matrix (contraction-d sharded over
      cores, host sums; fp32 -- selection-critical), alpha/beta =
      emb @ W0a/b h-slices in bf16 (value path).
  host: visibility mask + top_score add, per-row stable top-k -> ante_idx,
      distance buckets.
  L3: pair FFNN slow score, pairs sharded over cores (38 i-rows x 50 each).
      gamma term via fp8 DoubleRow matmuls on zT = ejT * broadcast(eiT);
      ej gathered bf16 + DMA-transposed; alpha/beta/dist folded into the
      same PSUM accumulation via small bf16 selection matmuls; final
      relu+w1+reduce in one DVE scalar_tensor_tensor with accum_out.

All selection-critical arithmetic (L1 scores, L2 pair matrix) is fp32;
the slow-score value path uses bf16/fp8 (tolerates >1e-2 noise).
"""
import numpy as np
import ml_dtypes
from contextlib import ExitStack

import concourse.bass as bass
import concourse.mybir as mybir
import concourse.tile as tile
from concourse import bacc
from concourse.bass_utils import run_bass_kernel_spmd
from concourse.masks import make_identity

dt = mybir.dt
AF = mybir.ActivationFunctionType
ALU = mybir.AluOpType
PM = mybir.MatmulPerfMode

NCORES = 8
T, E, W, F, H = 1536, 768, 16, 20, 1000
D = 2 * E + F + E              # 2324
DP = 2432                      # D padded to 19*128 (order: start,end,head,width,pad)
KC = DP // 128                 # 19 d-chunks
DP8 = 2560                     # L3 pair-term d padded to 20*128 (for fp8 DoubleRow)
KC8 = DP8 // 128               # 20
TC = T // NCORES               # 192 starts per core
COLS = 256                     # per-core t-window (t0 .. t0+256), te padded to 1600
TPAD = T + 64                  # 1600
SPC = TC * W                   # 3072 spans per core
M_CAP, K_CAP = 300, 50
MP = 304                       # m padded to 8*38
IPC = MP // NCORES             # 38 i-rows per core
PPC = IPC * K_CAP              # 1900 pairs per core
PPCB = 1920                    # padded to 15*128 tile blocks
PCH = PPCB // 128              # 15 pair chunks
S3 = 64.0                      # L3 value-path scale (fp8 range), undone via w1
NEG = np.float32(-1e30)

# se' feature permutation: old order [start(768) end(768) width(20) head(768)],
# new order [start end head width pad(108)]
_PERM = np.concatenate([
    np.arange(0, 2 * E),                   # start, end
    np.arange(2 * E + F, 2 * E + F + E),   # head
    np.arange(2 * E, 2 * E + F),           # width
]).astype(np.int64)            # maps new[0:2324] -> old index

_BIAS_C2F = 2430               # emb slot fixed to 1.0 (c2f bias row)
_BIAS_B0 = 2431                # emb slot fixed to 1.0 (Smm_b0 row via w0b)


def _pad_rows(a, rows):
    out = np.zeros((rows,) + a.shape[1:], a.dtype)
    out[:a.shape[0]] = a
    return out


# ---------------------------------------------------------------- L1 ----
def build_l1():
    nc = bacc.Bacc(trn_type="TRN2", target_bir_lowering=False, debug=False)
    te_loc = nc.dram_tensor("te_loc", [COLS, E], dt.float32, kind="ExternalInput").ap()
    p_loc = nc.dram_tensor("p_loc", [1, COLS], dt.float32, kind="ExternalInput").ap()
    r_loc = nc.dram_tensor("r_loc", [1, SPC], dt.float32, kind="ExternalInput").ap()
    w0abd = nc.dram_tensor("w0abd", [8, 18, 128, H // 8], dt.float32, kind="ExternalInput").ap()
    w0wd = nc.dram_tensor("w0wd", [F + 1, H], dt.float32, kind="ExternalInput").ap()
    we_aug = nc.dram_tensor("we_aug", [F + 1, W], dt.float32, kind="ExternalInput").ap()
    w1_in = nc.dram_tensor("w1_in", [H, 1], dt.float32, kind="ExternalInput").ap()
    score_out = nc.dram_tensor("score", [1, SPC], dt.float32, kind="ExternalOutput").ap()

    HC = 8
    HB = H // HC  # 125
    NT = 512

    with tile.TileContext(nc) as tc, ExitStack() as ctx:
        cst = ctx.enter_context(tc.tile_pool(name="cst", bufs=1))
        ps = ctx.enter_context(tc.tile_pool(name="ps", bufs=2, space="PSUM"))
        scps = ctx.enter_context(tc.tile_pool(name="scps", bufs=1, space="PSUM"))
        npool = ctx.enter_context(tc.tile_pool(name="npool", bufs=3))
        abp = ctx.enter_context(tc.tile_pool(name="abp", bufs=3))
        comb = ctx.enter_context(tc.tile_pool(name="comb", bufs=3))

        # --- constant loads (G-part of W first so PE starts early)
        te_sb = cst.tile([128, 2, E], dt.float32, tag="te_sb")
        nc.sync.dma_start(out=te_sb[:], in_=te_loc.rearrange("(c p) e -> p c e", p=128))
        ident = cst.tile([128, 128], dt.float32, tag="ident")
        make_identity(nc, ident)
        wdsb = cst.tile([F + 1, H], dt.float32, tag="wdsb")
        nc.sync.dma_start(out=wdsb[:], in_=w0wd[:, :])
        wesb = cst.tile([F + 1, W], dt.float32, tag="wesb")
        nc.sync.dma_start(out=wesb[:], in_=we_aug[:, :])
        w1sb = cst.tile([HB, HC], dt.float32, tag="w1sb")
        nc.sync.dma_start(out=w1sb[:], in_=w1_in.rearrange("(c p) one -> p (c one)", p=HB))
        p_bc = cst.tile([128, COLS], dt.float32, tag="p_bc")
        nc.sync.dma_start(out=p_bc[:], in_=p_loc[0:1, :].broadcast_to([128, COLS]))
        r_bc = cst.tile([128, SPC], dt.float32, tag="r_bc")
        nc.sync.dma_start(out=r_bc[:], in_=r_loc[0:1, :].broadcast_to([128, SPC]))

        # --- te^T tiles [128e, 256t] x6
        teT = []
        for ec in range(6):
            t_ = cst.tile([128, COLS], dt.float32, tag=f"teT{ec}", name=f"teT{ec}")
            teT.append(t_)
        for tcki in range(2):
            for ec in range(6):
                pt = ps.tile([128, 128], dt.float32, tag="acc")
                nc.tensor.transpose(out=pt[:], in_=te_sb[:, tcki, ec * 128:(ec + 1) * 128],
                                    identity=ident[:])
                nc.scalar.copy(teT[ec][:, tcki * 128:(tcki + 1) * 128], pt[:])

        sc_ps = [scps.tile([1, NT], dt.float32, tag=f"sc{j}", name=f"sc{j}")
                 for j in range(SPC // NT)]

        wpool = ctx.enter_context(tc.tile_pool(name="wpool", bufs=3))

        def mat_tile(whc, m3, tag):
            t_ = abp.tile([HB, COLS], dt.float32, tag=tag, name=tag)
            acc = ps.tile([HB, COLS], dt.float32, tag="acc", name="acc")
            for ec in range(6):
                nc.tensor.matmul(acc[:],
                                 lhsT=whc[:, m3 * 6 + ec, :],
                                 rhs=teT[ec][:],
                                 start=(ec == 0), stop=(ec == 5))
            nc.scalar.copy(t_[:], acc[:])
            return t_

        # --- per-h-chunk pipeline (W streamed per h-chunk, G chunks first)
        for hc in range(HC):
            whc = wpool.tile([128, 18, HB], dt.float32, tag="whc", name="whc")
            nc.sync.dma_start(out=whc[:], in_=w0abd[hc].rearrange("c p h -> p c h"))
            g_t = mat_tile(whc, 2, "g_t")
            pg = npool.tile([HB, COLS], dt.float32, tag="pg", name="pg")
            nc.vector.tensor_mul(pg[:], g_t[:], p_bc[:HB, :])
            numw = npool.tile([HB, SPC], dt.float32, tag="numw", name="numw")
            n3 = numw[:].rearrange("p (s w) -> p s w", w=W)
            nc.gpsimd.tensor_copy(n3[:, :, 0], pg[:, 0:TC])
            for w_ in range(1, W):
                nc.gpsimd.tensor_add(n3[:, :, w_], n3[:, :, w_ - 1], pg[:, w_:w_ + TC])
            a_t = mat_tile(whc, 0, "a_t")
            b_t = mat_tile(whc, 1, "b_t")
            wd_ps = ps.tile([HB, W], dt.float32, tag="acc", name="wdacc")
            nc.tensor.matmul(wd_ps[:], lhsT=wdsb[:, hc * HB:(hc + 1) * HB],
                             rhs=wesb[:], start=True, stop=True)
            wd_t = abp.tile([HB, W], dt.float32, tag="wd_t", name="wd_t")
            nc.scalar.copy(wd_t[:], wd_ps[:])

            # full-width combine: x = numW*r + A + B  (three wide DVE ops)
            x = comb.tile([HB, SPC], dt.float32, tag="x", name="x")
            nc.vector.tensor_mul(x[:], numw[:], r_bc[:HB, :])
            x3 = x[:].rearrange("p (s w) -> p s w", w=W)
            a_op = a_t[:, 0:TC][:, :, None].broadcast_to([HB, TC, W])
            nc.vector.tensor_add(x3, x3, a_op)
            bt = b_t[:]
            b_op = bass.AP(bt.tensor, bt.offset, [bt.ap[0], [1, TC], [1, W]])
            nc.vector.tensor_add(x3, x3, b_op)
            # wd folded into relu bias, one ACT op per width
            y = comb.tile([HB, SPC], dt.float32, tag="y", name="y")
            y3 = y[:].rearrange("p (s w) -> p s w", w=W)
            for w_ in range(W):
                nc.scalar.activation(y3[:, :, w_], x3[:, :, w_], AF.Relu,
                                     bias=wd_t[:, w_:w_ + 1])
            for j in range(SPC // NT):
                nc.tensor.matmul(sc_ps[j][:], lhsT=w1sb[:, hc:hc + 1],
                                 rhs=y[:, j * NT:(j + 1) * NT],
                                 start=(hc == 0), stop=(hc == HC - 1))

        for j in range(SPC // NT):
            so = comb.tile([1, NT], dt.float32, tag="so", name="so")
            nc.vector.tensor_copy(so[:], sc_ps[j][:])
            nc.sync.dma_start(out=score_out[:, j * NT:(j + 1) * NT], in_=so[:])

    nc.compile()
    return nc


# ---------------------------------------------------------------- L2 ----
def build_l2():
    nc = bacc.Bacc(trn_type="TRN2", target_bir_lowering=False, debug=False)
    embt_in = nc.dram_tensor("embt_in", [DP, M_CAP], dt.float32, kind="ExternalInput").ap()
    esel_in = nc.dram_tensor("esel_in", [128, 3, M_CAP], dt.float32, kind="ExternalInput").ap()
    c2f_c = nc.dram_tensor("c2f_c", [DP, 384], dt.float32, kind="ExternalInput").ap()
    w0a_c = nc.dram_tensor("w0a_c", [DP, 125], dt.bfloat16, kind="ExternalInput").ap()
    w0b_c = nc.dram_tensor("w0b_c", [DP, 125], dt.bfloat16, kind="ExternalInput").ap()

    pair_out = nc.dram_tensor("pair_part", [M_CAP, M_CAP], dt.float32, kind="ExternalOutput").ap()
    a_out = nc.dram_tensor("alpha_part", [M_CAP, 125], dt.bfloat16, kind="ExternalOutput").ap()
    b_out = nc.dram_tensor("beta_part", [M_CAP, 125], dt.bfloat16, kind="ExternalOutput").ap()

    MM = M_CAP
    with tile.TileContext(nc) as tc, ExitStack() as ctx:
        cst = ctx.enter_context(tc.tile_pool(name="cst", bufs=1))
        ps = ctx.enter_context(tc.tile_pool(name="ps", bufs=2, space="PSUM"))

        ident = cst.tile([128, 128], dt.float32, tag="ident")
        make_identity(nc, ident)
        embT = cst.tile([128, KC, MM], dt.float32, tag="embT")
        embT16 = cst.tile([128, KC, MM], dt.bfloat16, tag="embT16")
        c2f_sb = cst.tile([128, KC, 384], dt.float32, tag="c2f_sb")
        embt3 = embt_in.rearrange("(c p) s -> p c s", p=128)
        c2f3 = c2f_c.rearrange("(c p) s -> p c s", p=128)
        # chunked, queue-split loads so the src matmul chain starts on the
        # first chunk instead of a whole-tile barrier
        for ck in range(KC):
            nc.scalar.dma_start(out=embT[:, ck, :], in_=embt3[:, ck, :])
            nc.sync.dma_start(out=c2f_sb[:, ck, :], in_=c2f3[:, ck, :])
            nc.vector.tensor_copy(embT16[:, ck, :], embT[:, ck, :])
        esel = cst.tile([128, 3, MM], dt.float32, tag="esel")
        nc.scalar.dma_start(out=esel[:], in_=esel_in[:, :, :])
        w0a_sb = cst.tile([128, KC, 125], dt.bfloat16, tag="w0a_sb")
        nc.sync.dma_start(out=w0a_sb[:], in_=w0a_c.rearrange("(c p) s -> p c s", p=128))
        w0b_sb = cst.tile([128, KC, 125], dt.bfloat16, tag="w0b_sb")
        nc.sync.dma_start(out=w0b_sb[:], in_=w0b_c.rearrange("(c p) s -> p c s", p=128))

        # --- src slots + partial pair
        slotp = ctx.enter_context(tc.tile_pool(name="slotp", bufs=1))
        srcT = []
        for si in range(3):
            t_ = slotp.tile([128, MM], dt.float32, tag=f"srcT{si}", name=f"srcT{si}")
            srcT.append(t_)
        for si in range(3):
            acc = ps.tile([128, MM], dt.float32, tag="acc_ps")
            for kc in range(KC):
                nc.tensor.matmul(acc[:], lhsT=c2f_sb[:, kc, si * 128:(si + 1) * 128],
                                 rhs=embT[:, kc, :], start=(kc == 0), stop=(kc == KC - 1))
            nc.scalar.copy(srcT[si][:], acc[:])
        prow = ctx.enter_context(tc.tile_pool(name="prow", bufs=2))
        for it in range(3):
            inn = min(128, MM - it * 128)
            acc = ps.tile([128, MM], dt.float32, tag="acc_ps")
            for si in range(3):
                nc.tensor.matmul(acc[:inn, :], lhsT=srcT[si][:, it * 128:it * 128 + inn],
                                 rhs=esel[:, si, :], start=(si == 0), stop=(si == 2))
            pr = prow.tile([128, MM], dt.float32, tag="pr")
            nc.scalar.copy(pr[:inn, :], acc[:inn, :])
            nc.sync.dma_start(out=pair_out[it * 128:it * 128 + inn, :], in_=pr[:inn, :])

        # --- alpha/beta h-chunk (125 cols of H per core), bf16 value path
        abrow = ctx.enter_context(tc.tile_pool(name="abrow", bufs=2))
        for wsb_, out_ in ((w0a_sb, a_out), (w0b_sb, b_out)):
            acc = ps.tile([125, MM], dt.float32, tag="acc_ps")
            for kc in range(KC):
                nc.tensor.matmul(acc[:], lhsT=wsb_[:, kc, :], rhs=embT16[:, kc, :],
                                 start=(kc == 0), stop=(kc == KC - 1))
            abT = abrow.tile([125, MM], dt.float32, tag="abT")
            nc.scalar.copy(abT[:], acc[:])
            for g in range(3):
                gn = min(128, MM - g * 128)
                pt = ps.tile([128, 125], dt.float32, tag="tr_ps")
                nc.tensor.transpose(out=pt[:gn, :], in_=abT[:, g * 128:g * 128 + gn],
                                    identity=ident[:125, :125])
                ab_r = abrow.tile([128, 125], dt.bfloat16, tag="ab_r")
                nc.scalar.copy(ab_r[:gn, :], pt[:gn, :])
                nc.sync.dma_start(out=out_[g * 128:g * 128 + gn, :], in_=ab_r[:gn, :])

    nc.compile()
    return nc


# ---------------------------------------------------------------- L3 ----
def build_l3():
    nc = bacc.Bacc(trn_type="TRN2", target_bir_lowering=False, debug=False)
    embb = nc.dram_tensor("embb", [MP, DP8], dt.bfloat16, kind="ExternalInput").ap()
    jidx = nc.dram_tensor("jidx", [PPCB, 1], dt.int32, kind="ExternalInput").ap()
    # one bf16 constant blob [128, BLOB]: eit | jind | beta | dac | w0da | w1
    # (dac/w0da use partitions 0:58; w1/S replicated across partitions)
    BL_EIT = 0
    BL_JIND = BL_EIT + KC8 * IPC
    BL_BETA = BL_JIND + 3 * PPCB
    BL_DAC = BL_BETA + 3 * H
    BL_W0DA = BL_DAC + PPCB
    BL_W1 = BL_W0DA + H
    BLOB = BL_W1 + H
    blob_in = nc.dram_tensor("blob_in", [128, BLOB], dt.bfloat16, kind="ExternalInput").ap()
    w0c_in = nc.dram_tensor("w0c_in", [128, KC8, H], dt.float8e4, kind="ExternalInput").ap()
    slow_out = nc.dram_tensor("slow", [128, PCH], dt.float32, kind="ExternalOutput").ap()

    with tile.TileContext(nc) as tc, ExitStack() as ctx:
        cst = ctx.enter_context(tc.tile_pool(name="cst", bufs=1))
        jidx_sb = cst.tile([128, PCH], dt.int32, tag="jidx_sb")
        nc.sync.dma_start(out=jidx_sb[:], in_=jidx.rearrange("(c p) one -> p (c one)", p=128))
        blob = cst.tile([128, BLOB], dt.bfloat16, tag="blob")
        nc.scalar.dma_start(out=blob[:], in_=blob_in[:, :])
        eit_sb = blob[:, BL_EIT:BL_JIND].rearrange("p (c i) -> p c i", c=KC8)
        jind_sb = blob[:, BL_JIND:BL_BETA].rearrange("p (g s) -> p g s", g=3)
        beta_sb = blob[:, BL_BETA:BL_DAC].rearrange("p (g h) -> p g h", g=3)
        dac_sb = blob[:F + IPC, BL_DAC:BL_W0DA]
        w0da_sb = blob[:F + IPC, BL_W0DA:BL_W1]
        w1_bc = blob[:, BL_W1:BL_W1 + H]
        w0c_sb = cst.tile([128, KC8, H], dt.float8e4, tag="w0c_sb")
        nc.sync.dma_start(out=w0c_sb[:], in_=w0c_in[:, :, :])
        slow_sb = cst.tile([128, PCH], dt.float32, tag="slow_sb")

        ejT = cst.tile([128, KC8, PPCB], dt.bfloat16, tag="ejT")
        zT = cst.tile([128, KC8, PPCB], dt.float8e4, tag="zT")
        # pad pair columns (no eiT data there): zero once
        nc.vector.memset(zT[:, :, PPC:PPCB], 0.0)

        wk = ctx.enter_context(tc.tile_pool(name="wk", bufs=4))
        ps = ctx.enter_context(tc.tile_pool(name="ps", bufs=8, space="PSUM"))

        NH = H // 2
        half_acc = [cst.tile([128, PCH], dt.float32, tag=f"hacc{hh}", name=f"hacc{hh}")
                    for hh in range(2)]

        def mm_group(pc, hh):
            # (delta+alpha) + beta + gamma in one PSUM group (H in 500-wide
            # halves: matmul output must fit one PSUM bank), then
            # relu * w1 + reduce via stt accum_out
            blk = slice(pc * 128, (pc + 1) * 128)
            hs = slice(hh * NH, (hh + 1) * NH)
            acc = ps.tile([128, NH], dt.float32, tag="acc", name="acc")
            nc.tensor.matmul(acc[:], lhsT=dac_sb[:, blk], rhs=w0da_sb[:, hs],
                             start=True, stop=False, skip_group_check=True)
            for g in range(3):
                nc.tensor.matmul(acc[:], lhsT=jind_sb[:, g, blk],
                                 rhs=beta_sb[:, g, hs],
                                 start=False, stop=False, skip_group_check=True)
            for g in range(KC8 // 2):
                nc.tensor.matmul(acc[:], lhsT=zT[:, 2 * g:2 * g + 2, blk],
                                 rhs=w0c_sb[:, 2 * g:2 * g + 2, hs],
                                 perf_mode=PM.DoubleRow,
                                 start=False, stop=(g == KC8 // 2 - 1),
                                 skip_group_check=True)
            y = wk.tile([128, NH], dt.bfloat16, tag="y", name="y")
            nc.vector.scalar_tensor_tensor(
                y[:], acc[:], 0.0, w1_bc[:, hs],
                ALU.max, ALU.mult, accum_out=half_acc[hh][:, pc:pc + 1])

        # pipeline: gather+transpose the pair blocks a 500-pair span needs,
        # z-mul that span (all d-chunks), then issue the matmul groups the
        # previous span enabled (keeps stt's interleaved in the DVE queue so
        # PSUM banks recycle while later spans are still multiplying).
        ZCH = 500  # 10 i-groups of 50
        zlims = list(range(0, PPC, ZCH)) + [PPC]
        done_pc = 0
        grp_done = 0
        for zi in range(len(zlims) - 1):
            a, b = zlims[zi], zlims[zi + 1]
            need_pc = (b - 1) // 128
            for pc in range(done_pc, need_pc + 1):
                ej = wk.tile([128, DP8], dt.bfloat16, tag="ej", name="ej")
                nc.gpsimd.indirect_dma_start(
                    out=ej[:], out_offset=None, in_=embb[:, :],
                    in_offset=bass.IndirectOffsetOnAxis(ap=jidx_sb[:, pc:pc + 1], axis=0))
                eng = nc.sync if pc % 2 == 0 else nc.scalar
                eng.dma_start_transpose(ejT[:, :, pc * 128:(pc + 1) * 128], ej[:])
            done_pc = need_pc + 1
            ia, ib = a // K_CAP, b // K_CAP
            for kc in range(KC8):
                o3 = zT[:, kc, a:b].rearrange("p (i k) -> p i k", k=K_CAP)
                e3 = ejT[:, kc, a:b].rearrange("p (i k) -> p i k", k=K_CAP)
                w3 = eit_sb[:, kc, ia:ib][:, :, None].broadcast_to(
                    [128, ib - ia, K_CAP])
                nc.vector.tensor_mul(o3, e3, w3)
            # pair blocks fully covered by spans processed so far
            ready_pc = b // 128
            for pc in range(grp_done, ready_pc):
                mm_group(pc, 0)
                mm_group(pc, 1)
            grp_done = ready_pc
        for pc in range(grp_done, PCH):
            mm_group(pc, 0)
            mm_group(pc, 1)
        nc.vector.tensor_add(slow_sb[:], half_acc[0][:], half_acc[1][:])
        nc.sync.dma_start(out=slow_out[:, :], in_=slow_sb[:])

    nc.compile()
    return nc


# ------------------------------------------------------------- host -----
_BUILT = {}
TRACE = False
PERF = {}


def _run(name, nc, in_maps, cores):
    try:
        res = run_bass_kernel_spmd(nc, in_maps, core_ids=cores, trace=TRACE)
    except ModuleNotFoundError:
        res = run_bass_kernel_spmd(nc, in_maps, core_ids=cores)
    if res.exec_time_ns is not None:
        PERF[name] = res.exec_time_ns
    return res


def _get(name, builder):
    if name not in _BUILT:
        _BUILT[name] = builder()
    return _BUILT[name]


def _bucket_dist(d):
    df = d.astype(np.float32)
    with np.errstate(divide="ignore"):
        log_idx = np.floor(np.log(np.maximum(df, np.float32(1.0))) /
                           np.float32(np.log(2.0))) + np.float32(3.0)
    comb = np.where(d <= 4, df, log_idx)
    return np.clip(comb, 0, 9).astype(np.int32)


def _nms(order, starts, ends, m):
    top = []
    ts = np.zeros(m, np.int64)
    te_ = np.zeros(m, np.int64)
    n = 0
    i = 0
    C = order.shape[0]
    while n < m and i < C:
        idx = order[i]
        s, e = starts[idx], ends[idx]
        if n:
            a = ts[:n]
            b = te_[:n]
            cross = (((s < a) & (e < b) & (e >= a)) | ((s > a) & (s <= b) & (e > b))).any()
        else:
            cross = False
        if not cross:
            top.append(idx)
            ts[n] = s
            te_[n] = e
            n += 1
        i += 1
    return np.asarray(top, np.int64)


def kernel(tokens_embed, Wh_w, Wh_b, width_emb, dist_emb,
           Sm_w0, Sm_b0, Sm_w1, Sm_b1,
           Smm_w0, Smm_b0, Smm_w1, Smm_b1,
           c2f_w, c2f_b, m, k):
    m = int(m)
    k = int(k)
    assert m == M_CAP and k == K_CAP, (m, k)
    f32 = np.float32
    bf16 = ml_dtypes.bfloat16
    f8 = ml_dtypes.float8_e4m3fn
    te = np.ascontiguousarray(tokens_embed, f32)
    cores = list(range(NCORES))

    # ---- host prep (index/pad/exp glue)
    te_pad = np.concatenate([te, np.repeat(te[-1:], TPAD - T, 0)], 0)
    tok = (te @ Wh_w.astype(f32))[:, 0] + f32(Wh_b[0])
    p_full = np.exp(tok.astype(f32)).astype(f32)
    p_pad = np.concatenate([p_full, np.zeros(TPAD - T, f32)])
    # den/r per (s, w): sequential fp32 prefix
    den = np.empty((T, W), f32)
    acc = p_full.copy()
    den[:, 0] = acc
    for w_ in range(1, W):
        nxt = np.concatenate([p_full[w_:], np.zeros(w_, f32)])
        acc = (acc + nxt).astype(f32)
        den[:, w_] = acc
    r_full = (f32(1.0) / den).astype(f32)
    # cumulative p*te for the head segment sums: cum[t] = sum_{t'<t} p*te
    cum_pte = np.zeros((T + 1, E), f32)
    np.cumsum((p_full[:, None] * te).astype(f32), axis=0, dtype=f32,
              out=cum_pte[1:])

    # L1 inputs
    w0abd_flat = np.concatenate([Sm_w0[0:E], Sm_w0[E:2 * E], Sm_w0[2 * E + F:]], 0).astype(f32)
    w0abd = np.ascontiguousarray(
        w0abd_flat.reshape(18, 128, 8, 125).transpose(2, 0, 1, 3))
    w0wd = np.concatenate([Sm_w0[2 * E:2 * E + F], Sm_b0[None, :]], 0).astype(f32)
    we_aug = np.concatenate([width_emb.T, np.ones((1, W))], 0).astype(f32)
    in1 = []
    for c in cores:
        t0 = c * TC
        in1.append(dict(
            te_loc=np.ascontiguousarray(te_pad[t0:t0 + COLS]),
            p_loc=np.ascontiguousarray(p_pad[None, t0:t0 + COLS]),
            r_loc=np.ascontiguousarray(r_full[t0:t0 + TC].reshape(1, SPC)),
            w0abd=w0abd, w0wd=w0wd, we_aug=we_aug,
            w1_in=Sm_w1.astype(f32)))
    nc1 = _get("l1", build_l1)
    res1 = _run("l1", nc1, in1, cores)
    scores = np.concatenate([res1.results[c]["score"][0] for c in cores])

    # ---- host: mask + NMS
    starts = np.repeat(np.arange(T, dtype=np.int64), W)
    widths = np.tile(np.arange(W, dtype=np.int64), T)
    ends = starts + widths
    valid = ends < T
    sc = np.where(valid, (scores + f32(Sm_b1[0])).astype(f32), NEG).astype(f32)
    order = np.argsort(-sc, kind="stable")
    top_idx = _nms(order, starts, ends, m)
    ts_, tw = starts[top_idx], widths[top_idx]
    tec = np.minimum(ts_ + tw, T - 1)
    top_score = sc[top_idx]

    # ---- host: span embedding assembly (indexing glue)
    head = ((cum_pte[tec + 1] - cum_pte[ts_]) *
            r_full[ts_, tw][:, None]).astype(f32)
    emb = np.zeros((M_CAP, DP), f32)
    emb[:, 0:E] = te[ts_]
    emb[:, E:2 * E] = te[tec]
    emb[:, 2 * E:3 * E] = head
    emb[:, 3 * E:3 * E + F] = width_emb.astype(f32)[tw]
    emb[:, _BIAS_C2F] = 1.0
    emb[:, _BIAS_B0] = 1.0
    embt_in = np.ascontiguousarray(emb.T)

    # ---- L2
    perm = _PERM
    c2f_pad = np.zeros((DP, DP), f32)
    c2f_pad[:D, :D] = c2f_w.astype(f32)[np.ix_(perm, perm)]
    c2f_pad[_BIAS_C2F, :D] = c2f_b.astype(f32)[perm]
    w0a_pad = np.zeros((DP, H), f32)
    w0a_pad[:D] = Smm_w0[0:D].astype(f32)[perm] * f32(S3)
    w0b_pad = np.zeros((DP, H), f32)
    w0b_pad[:D] = Smm_w0[D:2 * D].astype(f32)[perm] * f32(S3)
    w0b_pad[_BIAS_B0] = Smm_b0.astype(f32) * f32(S3)
    in2 = []
    for c in cores:
        hc0 = c * 125
        cs = [3 * c, 3 * c + 1, 3 * c + 2]
        c2fc = np.zeros((DP, 384), f32)
        eselc = np.zeros((128, 3, M_CAP), f32)
        for si, ch in enumerate(cs):
            if ch < KC:
                c2fc[:, si * 128:(si + 1) * 128] = c2f_pad[:, ch * 128:(ch + 1) * 128]
                eselc[:, si, :] = embt_in[ch * 128:(ch + 1) * 128, :]
        in2.append(dict(
            embt_in=embt_in, esel_in=eselc, c2f_c=c2fc,
            w0a_c=np.ascontiguousarray(w0a_pad[:, hc0:hc0 + 125]).astype(bf16),
            w0b_c=np.ascontiguousarray(w0b_pad[:, hc0:hc0 + 125]).astype(bf16)))
    nc2 = _get("l2", build_l2)
    res2 = _run("l2", nc2, in2, cores)
    pair = np.zeros((m, m), f32)
    for c in cores:
        pair += res2.results[c]["pair_part"]
    pair = pair.astype(f32)
    alpha16 = np.concatenate([res2.results[c]["alpha_part"] for c in cores], 1)
    beta16 = np.concatenate([res2.results[c]["beta_part"] for c in cores], 1)

    # ---- host: visibility + top-k
    offset = np.arange(m, dtype=np.int64)[:, None] - np.arange(m, dtype=np.int64)[None, :]
    vis = offset >= 1
    all_score = (np.where(vis, f32(0.0), NEG).astype(f32) + top_score[:, None]).astype(f32)
    all_score = (all_score + pair).astype(f32)
    ante_idx = np.argsort(-all_score, axis=1, kind="stable")[:, :k]
    fast = np.take_along_axis(all_score, ante_idx, axis=1).astype(f32)
    ante_off = np.take_along_axis(offset, ante_idx, axis=1)
    dbuck = _bucket_dist(ante_off)

    # ---- L3
    emb16 = np.zeros((MP, DP8), bf16)
    emb16[:m, :DP] = emb.astype(bf16)
    emb16_f = emb16.astype(f32)
    w0c_pad = np.zeros((DP8, H), f32)
    w0c_pad[:D] = Smm_w0[2 * D:3 * D].astype(f32)[perm] * f32(S3)
    w0c8 = np.ascontiguousarray(
        w0c_pad.reshape(KC8, 128, H).transpose(1, 0, 2)).astype(f8)
    w0d16 = (Smm_w0[3 * D:3 * D + F].astype(f32) * f32(S3)).astype(bf16)
    ind16 = np.zeros((IPC, PPCB), bf16)
    for r_ in range(IPC):
        ind16[r_, r_ * K_CAP:(r_ + 1) * K_CAP] = 1.0
    beta_pad = _pad_rows(np.asarray(beta16, bf16), 384)
    beta_c = np.ascontiguousarray(beta_pad.reshape(3, 128, H).transpose(1, 0, 2))
    alpha_pad = _pad_rows(np.asarray(alpha16, bf16), MP)
    in3 = []
    for c in cores:
        i0 = c * IPC
        jj = np.zeros(PPCB, np.int32)
        dmb = np.zeros((F, PPCB), f32)
        for r_ in range(IPC):
            gi = i0 + r_
            if gi < m:
                sl = slice(r_ * K_CAP, (r_ + 1) * K_CAP)
                jj[sl] = ante_idx[gi]
                dmb[:, sl] = dist_emb.astype(f32)[dbuck[gi]].T
        eit = np.ascontiguousarray(
            emb16_f[i0:i0 + IPC].reshape(IPC, KC8, 128).transpose(2, 1, 0)).astype(bf16)
        jind = np.zeros((128, 3, PPCB), bf16)
        jind[jj % 128, jj // 128, np.arange(PPCB)] = 1.0
        dac = np.concatenate([dmb.astype(bf16), ind16], 0)
        w0da = np.concatenate([w0d16, np.asarray(alpha_pad[i0:i0 + IPC])], 0)
        # bf16 constant blob: eit | jind | beta | dac | w0da | w1 (see build_l3)
        nda = F + IPC
        blob = np.zeros((128, KC8 * IPC + 3 * PPCB + 3 * H + PPCB + 2 * H), bf16)
        o = 0
        blob[:, o:o + KC8 * IPC] = eit.reshape(128, KC8 * IPC); o += KC8 * IPC
        blob[:, o:o + 3 * PPCB] = jind.reshape(128, 3 * PPCB); o += 3 * PPCB
        blob[:, o:o + 3 * H] = beta_c.reshape(128, 3 * H); o += 3 * H
        blob[:nda, o:o + PPCB] = dac; o += PPCB
        blob[:nda, o:o + H] = w0da; o += H
        blob[:, o:o + H] = (Smm_w1.astype(f32).T / f32(S3)).astype(bf16)
        in3.append(dict(
            embb=emb16, jidx=jj[:, None], blob_in=blob, w0c_in=w0c8))
    nc3 = _get("l3", build_l3)
    res3 = _run("l3", nc3, in3, cores)
    slow = np.zeros((m, k), f32)
    for c in cores:
        sl = np.ascontiguousarray(res3.results[c]["slow"].T).reshape(PPCB)
        i0 = c * IPC
        for r_ in range(IPC):
            gi = i0 + r_
            if gi < m:
                slow[gi] = sl[r_ * K_CAP:(r_ + 1) * K_CAP]
    slow = (slow + f32(Smm_b1[0])).astype(f32)
    return (fast + slow).astype(f32)
